# revision 14
# baseline (speedup 1.0000x reference)
"""MultiHeadAttention Trainium2 kernel (8 NeuronCores).

Sharding: core c handles batch b = c // 2 and head-group hg = c % 2
(8 of 16 heads, 512 of 1024 model dims). Attention is embarrassingly
parallel over (b, hg); the output projection is computed per head-group
against the matching W_o columns, yielding partial outputs that the host
sums (plus b_o).

Device dataflow (per core), all in "transposed" layouts so no on-device
transposes are ever needed:
  qT = Wq_hg @ Xq^T      [dh=512, S]   (lhsT = Wq_hg^T, rhs = Xq^T; host preps both)
  kT = Wk_hg @ Xk^T      [dh=512, S]
  v  = Xv @ Wv_hg^T      [S, dh=512]   (+ ones column per head for softmax sums)
  scores_T[k, q] = kT_h[:, kchunk]^T-matmul  (keys on partitions)
  causal mask: extra PE matmul tri^T @ step accumulating -1e9 into masked entries
  probs = exp(scores_T / 8) on ACT (no max subtraction: scores ~ N(0,1), safe)
  attn_T[d, q] (+ sums row) = v_chunk^T-matmul over probs, accumulated in PSUM
  normalize: recip = 1/sums (DVE), broadcast via ones-matmul, multiply (DVE)
  out_partial = attn^T-matmul with Wo columns
"""

import os

import numpy as np

B, S_FULL, D = 4, 2048, 1024
H, DK = 16, 64
NH_G = 8          # heads per core
DH = NH_G * DK    # 512 dims per core
P = 128
KC = 128          # key chunk (PE contraction)
NEG = -1.0e9
SCALE = 1.0 / np.sqrt(np.float32(DK))

_PROG_CACHE = {}


def _dims(S):
    QB = min(512, S)
    return {
        "S": S, "QB": QB, "N_QB": S // QB, "N_KC": S // KC,
        "R": QB // KC, "E_CH": D // P, "M_CH": DH // P, "O_N": D // 512,
    }


def _np_dt(use_bf16):
    if use_bf16:
        import ml_dtypes
        return ml_dtypes.bfloat16
    return np.float32


def build_program(causal, S, use_bf16, debug_dumps=False):
    """Build the single-core Bass/Tile program (same program on all 8 cores)."""
    from contextlib import ExitStack

    import concourse.bass as bass
    import concourse.tile as tile
    from concourse import bacc, mybir

    d = _dims(S)
    QB, N_QB, N_KC, R, E_CH, M_CH, O_N = (
        d["QB"], d["N_QB"], d["N_KC"], d["R"], d["E_CH"], d["M_CH"], d["O_N"])

    DT = mybir.dt.bfloat16 if use_bf16 else mybir.dt.float32r
    F32 = mybir.dt.float32
    F32R = mybir.dt.float32r
    AF = mybir.ActivationFunctionType
    ALU = mybir.AluOpType

    nc = bacc.Bacc("TRN2", target_bir_lowering=False, debug=False)

    xq_t = nc.dram_tensor("xq_t", [D, S], DT, kind="ExternalInput").ap()
    xk_t = nc.dram_tensor("xk_t", [D, S], DT, kind="ExternalInput").ap()
    xv_t = nc.dram_tensor("xv_t", [D, S], DT, kind="ExternalInput").ap()
    wq_t = nc.dram_tensor("wq_t", [D, DH], DT, kind="ExternalInput").ap()
    wk_t = nc.dram_tensor("wk_t", [D, DH], DT, kind="ExternalInput").ap()
    wv_t = nc.dram_tensor("wv_t", [D, DH], DT, kind="ExternalInput").ap()
    wo_t = nc.dram_tensor("wo_t", [DH, D], DT, kind="ExternalInput").ap()
    bq_in = nc.dram_tensor("bq_p", [P, M_CH], F32, kind="ExternalInput").ap()
    bk_in = nc.dram_tensor("bk_p", [P, M_CH], F32, kind="ExternalInput").ap()
    bv_in = nc.dram_tensor("bv_r", [P, DH], F32, kind="ExternalInput").ap()
    tri_in = nc.dram_tensor("tri", [P, KC], DT, kind="ExternalInput").ap()
    stepm_in = nc.dram_tensor("stepm", [P, R, QB], DT, kind="ExternalInput").ap()
    ones_c_in = nc.dram_tensor("ones_c", [65, 64], F32R,
                               kind="ExternalInput").ap()
    ones_v_in = nc.dram_tensor("ones_v", [P, N_KC, NH_G, 1], DT,
                               kind="ExternalInput").ap()
    out_p = nc.dram_tensor("out_p", [S, D], F32, kind="ExternalOutput").ap()
    if debug_dumps:
        dbg_qT = nc.dram_tensor("dbg_qT", [P, M_CH, S], DT,
                                kind="ExternalOutput").ap()
        dbg_kT = nc.dram_tensor("dbg_kT", [P, M_CH, S], DT,
                                kind="ExternalOutput").ap()
        dbg_vaug = nc.dram_tensor("dbg_vaug", [P, N_KC, NH_G, 65], DT,
                                  kind="ExternalOutput").ap()
        dbg_probs = nc.dram_tensor("dbg_probs", [P, 2 * QB], DT,
                                   kind="ExternalOutput").ap()
        dbg_attn = nc.dram_tensor("dbg_attn", [M_CH, P, QB], DT,
                                  kind="ExternalOutput").ap()
        dbg_recip = nc.dram_tensor("dbg_recip", [P, 3, QB], F32,
                                   kind="ExternalOutput").ap()

    with tile.TileContext(nc) as tc, ExitStack() as ctx:
        consts = ctx.enter_context(tc.tile_pool(name="consts", bufs=1))
        wpool = ctx.enter_context(tc.tile_pool(name="w", bufs=2))
        qkv = ctx.enter_context(tc.tile_pool(name="qkv", bufs=1))

        tri = consts.tile([P, KC], DT)
        nc.sync.dma_start(tri, tri_in)
        stepm = consts.tile([P, R, QB], DT)
        nc.sync.dma_start(stepm, stepm_in)
        bq_sb = consts.tile([P, M_CH], F32)
        nc.sync.dma_start(bq_sb, bq_in)
        bk_sb = consts.tile([P, M_CH], F32)
        nc.sync.dma_start(bk_sb, bk_in)
        bv_sb = consts.tile([P, DH], F32)
        nc.sync.dma_start(bv_sb, bv_in)
        ones65 = consts.tile([65, 64], F32R)
        nc.sync.dma_start(ones65, ones_c_in)

        qT = qkv.tile([P, M_CH, S], DT, tag="qT")
        kT = qkv.tile([P, M_CH, S], DT, tag="kT")
        v_aug = qkv.tile([P, N_KC, NH_G, 65], DT, tag="v_aug")
        nc.sync.dma_start(v_aug[:, :, :, 64:65], ones_v_in)

        w_tiles = {}
        for name, src in (("wq", wq_t), ("wk", wk_t), ("wv", wv_t)):
            w_sb = wpool.tile([P, E_CH, DH], DT, tag="w")
            nc.sync.dma_start(w_sb, src.rearrange("(eo p) m -> p eo m", p=P))
            w_tiles[name] = w_sb
        wo_sb = wpool.tile([P, M_CH, D], DT, tag="w")
        nc.sync.dma_start(wo_sb, wo_t.rearrange("(mo p) n -> p mo n", p=P))

        # ---- projections ----
        with tc.tile_pool(name="xp", bufs=3) as xpool, \
             tc.tile_pool(name="pj", bufs=3, space="PSUM") as pj_ps:
            for phase, x_in, w_sb, b_sb in (
                ("q", xq_t, w_tiles["wq"], bq_sb),
                ("k", xk_t, w_tiles["wk"], bk_sb),
                ("v", xv_t, w_tiles["wv"], bv_sb),
            ):
                dst = qT if phase == "q" else kT
                for n in range(N_QB):
                    xblk = xpool.tile([P, E_CH, QB], DT, tag="x")
                    nc.sync.dma_start(
                        xblk,
                        x_in.rearrange("(eo p) s -> p eo s", p=P)[
                            :, :, n * QB:(n + 1) * QB],
                    )
                    if phase in ("q", "k"):
                        for m in range(M_CH):
                            ps = pj_ps.tile([P, QB], F32, tag="pj")
                            for e in range(E_CH):
                                nc.tensor.matmul(
                                    ps,
                                    lhsT=w_sb[:, e, m * P:(m + 1) * P],
                                    rhs=xblk[:, e, :],
                                    start=(e == 0), stop=(e == E_CH - 1),
                                )
                            nc.vector.tensor_scalar_add(
                                dst[:, m, n * QB:(n + 1) * QB], ps,
                                b_sb[:, m:m + 1])
                    else:
                        for sc in range(QB // P):
                            ps = pj_ps.tile([P, DH], F32, tag="pj")
                            for e in range(E_CH):
                                nc.tensor.matmul(
                                    ps,
                                    lhsT=xblk[:, e, sc * P:(sc + 1) * P],
                                    rhs=w_sb[:, e, :],
                                    start=(e == 0), stop=(e == E_CH - 1),
                                )
                            kc = n * (QB // P) + sc
                            nc.vector.tensor_tensor(
                                v_aug[:, kc, :, 0:64],
                                ps.rearrange("p (h e) -> p h e", h=NH_G),
                                bv_sb.rearrange("p (h e) -> p h e", h=NH_G),
                                ALU.add,
                            )

        if debug_dumps:
            nc.sync.dma_start(dbg_qT, qT)
            nc.sync.dma_start(dbg_kT, kT)
            nc.sync.dma_start(dbg_vaug, v_aug)

        # ---- attention + output projection ----
        with tc.tile_pool(name="sc_ps", bufs=2, space="PSUM") as sc_ps, \
             tc.tile_pool(name="pv_ps", bufs=2, space="PSUM") as pv_pool, \
             tc.tile_pool(name="rb_ps", bufs=1, space="PSUM") as rb_pool, \
             tc.tile_pool(name="op_ps", bufs=1, space="PSUM") as op_ps, \
             tc.tile_pool(name="probs", bufs=4) as probs_pool, \
             tc.tile_pool(name="attn", bufs=M_CH + 1) as attn_pool, \
             tc.tile_pool(name="misc", bufs=3) as misc, \
             tc.tile_pool(name="outst", bufs=3) as outst:
            for qb in range(N_QB):
                attn_tiles = []
                for m in range(M_CH):
                    n_kc = (qb + 1) * (QB // KC) if causal else N_KC
                    pv_t = [pv_pool.tile([65, QB], F32, tag="pv", name=f"pv{hl}")
                             for hl in (0, 1)]
                    for pair in range(n_kc // 2):
                        ps_h = [sc_ps.tile([P, 2 * QB], F32, tag="sc",
                                          name=f"sc{hl}")
                                for hl in (0, 1)]
                        for dkc in (0, 1):
                            kc = 2 * pair + dkc
                            r = kc - (n_kc - R)
                            is_diag = causal and r >= 0
                            for hl in (0, 1):
                                rows = slice(64 * hl, 64 * hl + 64)
                                nc.tensor.matmul(
                                    ps_h[hl][:, dkc * QB:(dkc + 1) * QB],
                                    lhsT=kT[rows, m, kc * KC:(kc + 1) * KC],
                                    rhs=qT[rows, m, qb * QB:(qb + 1) * QB],
                                    start=True, stop=not is_diag,
                                )
                                if is_diag:
                                    nc.tensor.matmul(
                                        ps_h[hl][:, dkc * QB:(dkc + 1) * QB],
                                        lhsT=tri, rhs=stepm[:, r, :],
                                        start=False, stop=True,
                                    )
                        for hl in (0, 1):
                            pt = probs_pool.tile([P, 2 * QB], DT, tag="pt")
                            nc.scalar.activation(pt, ps_h[hl], AF.Exp,
                                                 scale=float(SCALE))
                            if (debug_dumps and qb == 0 and m == 0
                                    and pair == 0 and hl == 0):
                                nc.sync.dma_start(dbg_probs, pt)
                            for dkc in (0, 1):
                                kc = 2 * pair + dkc
                                nc.tensor.matmul(
                                    pv_t[hl],
                                    lhsT=v_aug[:, kc, 2 * m + hl, :],
                                    rhs=pt[:, dkc * QB:(dkc + 1) * QB],
                                    start=(kc == 0), stop=(kc == n_kc - 1),
                                )
                    attn_m = attn_pool.tile([P, QB], DT, tag="attn")
                    for hl in (0, 1):
                        # Drain PSUM fast (attn + sums copies) so the PV tile
                        # recycles without waiting on the slow reciprocal.
                        attn_u = misc.tile([64, QB], DT, tag="attn_u")
                        nc.any.tensor_copy(attn_u, pv_t[hl][0:64, :])
                        sums_sb = misc.tile([65, QB], F32, tag="sums_sb")
                        nc.any.tensor_copy(sums_sb[64:65, :],
                                           pv_t[hl][64:65, :])
                        recip65 = misc.tile([65, QB], F32R, tag="recip")
                        with nc.allow_low_precision(
                                reason="softmax denom recip, f32r rounding"):
                            nc.vector.reciprocal(recip65[64:65, :],
                                                 sums_sb[64:65, :])
                        rb = rb_pool.tile([64, QB], F32, tag="rb")
                        nc.tensor.matmul(rb, lhsT=ones65[64:65, :],
                                         rhs=recip65[64:65, :],
                                         start=True, stop=True)
                        if debug_dumps and qb == 0 and m == 0 and hl == 0:
                            nc.sync.dma_start(
                                dbg_recip[64:65, 1, :],
                                recip65[64:65, :].bitcast(F32))
                            nc.sync.dma_start(
                                dbg_recip[64:65, 0, :],
                                recip65[64:65, :].bitcast(F32))
                            rbsb = misc.tile([64, QB], F32, tag="rbsb",
                                             name="rbsb")
                            nc.vector.tensor_copy(rbsb, rb)
                            nc.sync.dma_start(dbg_recip[0:64, 2, :], rbsb)
                        nc.vector.tensor_tensor(
                            attn_m[64 * hl:64 * hl + 64, :], attn_u, rb,
                            ALU.mult)
                    if debug_dumps and qb == 0:
                        nc.sync.dma_start(dbg_attn[m], attn_m)
                    attn_tiles.append(attn_m)
                for ssub in range(QB // P):
                    for nout in range(O_N):
                        pso = op_ps.tile([P, 512], F32, tag="op")
                        for m in range(M_CH):
                            nc.tensor.matmul(
                                pso,
                                lhsT=attn_tiles[m][:, ssub * P:(ssub + 1) * P],
                                rhs=wo_sb[:, m, nout * 512:(nout + 1) * 512],
                                start=(m == 0), stop=(m == M_CH - 1),
                            )
                        st = outst.tile([P, 512], F32, tag="st")
                        nc.any.tensor_copy(st, pso)
                        nc.sync.dma_start(
                            out_p[qb * QB + ssub * P: qb * QB + (ssub + 1) * P,
                                  nout * 512:(nout + 1) * 512],
                            st)
    nc.compile()
    return nc


def make_consts(S, use_bf16):
    """Host-built mask-bias matmul operands (tri, stepm)."""
    d = _dims(S)
    QB, R = d["QB"], d["R"]
    npdt = _np_dt(use_bf16)
    tri = np.zeros((P, KC), np.float32)
    for t in range(P):
        tri[t, t:] = 1.0
    stepm = np.zeros((P, R, QB), np.float32)
    for r in range(R):
        for j in range(QB):
            c = j - KC * r
            if c >= KC - 1:
                continue
            stepm[max(0, c + 1), r, j] = NEG
    return tri.astype(npdt), stepm.astype(npdt)


def core_inputs(Q, K, V, W_q, b_q, W_k, b_k, W_v, b_v, W_o, b, hg, S, use_bf16):
    """Build the per-core input map (host-side slicing/transposition/casts)."""
    npdt = _np_dt(use_bf16)
    d = _dims(S)
    M_CH = d["M_CH"]
    rows = slice(hg * DH, (hg + 1) * DH)

    def t(x):
        return np.ascontiguousarray(np.asarray(x, np.float32).T).astype(npdt)

    tri, stepm = make_consts(S, use_bf16)
    return {
        "xq_t": t(Q[b]), "xk_t": t(K[b]), "xv_t": t(V[b]),
        "wq_t": t(W_q[rows]), "wk_t": t(W_k[rows]), "wv_t": t(W_v[rows]),
        "wo_t": t(W_o[:, rows]),
        "bq_p": np.ascontiguousarray(
            np.asarray(b_q[rows], np.float32).reshape(M_CH, P).T),
        "bk_p": np.ascontiguousarray(
            np.asarray(b_k[rows], np.float32).reshape(M_CH, P).T),
        "bv_r": np.broadcast_to(
            np.asarray(b_v[rows], np.float32), (P, DH)).copy(),
        "tri": tri, "stepm": stepm,
        "ones_c": np.ones((65, 64), np.float32),
        "ones_v": np.ones((P, d["N_KC"], NH_G, 1), npdt),
    }


def _np_reference(Q, K, V, mask, W_q, b_q, W_k, b_k, W_v, b_v, W_o, b_o):
    """Exact numpy fallback for arbitrary masks."""
    q = (Q @ W_q.T + b_q).reshape(B, S_FULL, H, DK).transpose(0, 2, 1, 3)
    k = (K @ W_k.T + b_k).reshape(B, S_FULL, H, DK).transpose(0, 2, 1, 3)
    v = (V @ W_v.T + b_v).reshape(B, S_FULL, H, DK).transpose(0, 2, 1, 3)
    scores = np.einsum("bhqd,bhkd->bhqk", q, k) / np.sqrt(np.float32(DK))
    scores = np.where(mask == 0, np.finfo(np.float32).min, scores)
    scores -= scores.max(-1, keepdims=True)
    probs = np.exp(scores)
    probs /= probs.sum(-1, keepdims=True)
    out = np.einsum("bhqk,bhkd->bhqd", probs, v)
    out = out.transpose(0, 2, 1, 3).reshape(B, S_FULL, D)
    return (out @ W_o.T + b_o).astype(np.float32)


def kernel(Q, K, V, mask, W_q, b_q, W_k, b_k, W_v, b_v, W_o, b_o):
    Q = np.asarray(Q, np.float32)
    K = np.asarray(K, np.float32)
    V = np.asarray(V, np.float32)
    mask = np.asarray(mask)

    m2 = mask.reshape(mask.shape[-2], mask.shape[-1])
    if np.array_equal(m2 != 0, np.tril(np.ones(m2.shape, bool))):
        causal = True
    elif (m2 != 0).all():
        causal = False
    else:
        return _np_reference(Q, K, V, mask, W_q, b_q, W_k, b_k, W_v, b_v,
                             W_o, b_o)

    use_bf16 = os.environ.get("MHA_KERNEL_DTYPE", "f32r") == "bf16"
    import concourse.bass_utils as _bu
    from concourse.bass_utils import run_bass_kernel_spmd
    if (os.environ.get("MHA_LDW_OPT", "0") == "1"
            and not getattr(_bu, "_mha_ldw_patched", False)):
        _orig_rc = _bu.run_command

        def _rc(argv, **kw):
            argv = ["--enable-ldw-opt=true" if a == "--enable-ldw-opt=false"
                    else a for a in argv]
            return _orig_rc(argv, **kw)

        _bu.run_command = _rc
        _bu._mha_ldw_patched = True

    key = (causal, S_FULL, use_bf16)
    if key not in _PROG_CACHE:
        _PROG_CACHE[key] = build_program(causal, S_FULL, use_bf16)
    nc = _PROG_CACHE[key]

    in_maps = []
    for c in range(8):
        b, hg = divmod(c, 2)
        in_maps.append(core_inputs(Q, K, V, W_q, b_q, W_k, b_k, W_v, b_v,
                                   W_o, b, hg, S_FULL, use_bf16))

    trace = os.environ.get("MHA_KERNEL_TRACE", "0") == "1"
    kw = {}
    if trace:
        kw = {"trace": True,
              "trace_cores": [int(x) for x in os.environ.get(
                  "MHA_TRACE_CORES", "0").split(",")]}
    res = run_bass_kernel_spmd(nc, in_maps, core_ids=list(range(8)), **kw)
    kernel.last_results = res

    b_o32 = np.asarray(b_o, np.float32)
    out = np.empty((B, S_FULL, D), np.float32)
    for b in range(B):
        out[b] = (res.results[2 * b]["out_p"] + res.results[2 * b + 1]["out_p"]
                  + b_o32[None, :])
    return out


kernel.last_results = None


# revision 16
# speedup vs baseline: 1.1512x; 1.1512x over previous
"""MultiHeadAttention Trainium2 kernel (8 NeuronCores).

Sharding: core c handles batch b = c // 2 and head-group hg = c % 2
(8 of 16 heads, 512 of 1024 model dims). Attention is embarrassingly
parallel over (b, hg); the output projection is computed per head-group
against the matching W_o columns, yielding partial outputs that the host
sums (plus b_o).

Device dataflow (per core), all in "transposed" layouts so no on-device
transposes are ever needed:
  qT = Wq_hg @ Xq^T      [dh=512, S]   (lhsT = Wq_hg^T, rhs = Xq^T; host preps both)
  kT = Wk_hg @ Xk^T      [dh=512, S]
  v  = Xv @ Wv_hg^T      [S, dh=512]   (+ ones column per head for softmax sums)
  scores_T[k, q] = kT_h[:, kchunk]^T-matmul  (keys on partitions)
  causal mask: extra PE matmul tri^T @ step accumulating -1e9 into masked entries
  probs = exp(scores_T / 8) on ACT (no max subtraction: scores ~ N(0,1), safe)
  attn_T[d, q] (+ sums row) = v_chunk^T-matmul over probs, accumulated in PSUM
  normalize: recip = 1/sums (DVE), broadcast via ones-matmul, multiply (DVE)
  out_partial = attn^T-matmul with Wo columns
"""

import os

import numpy as np

B, S_FULL, D = 4, 2048, 1024
H, DK = 16, 64
NH_G = 8          # heads per core
DH = NH_G * DK    # 512 dims per core
P = 128
KC = 128          # key chunk (PE contraction)
NEG = -1.0e9
SCALE = 1.0 / np.sqrt(np.float32(DK))

_PROG_CACHE = {}


def _dims(S):
    QB = min(512, S)
    return {
        "S": S, "QB": QB, "N_QB": S // QB, "N_KC": S // KC,
        "R": QB // KC, "E_CH": D // P, "M_CH": DH // P, "O_N": D // 512,
    }


def _np_dt(use_bf16):
    if use_bf16:
        import ml_dtypes
        return ml_dtypes.bfloat16
    return np.float32


def build_program(causal, S, use_bf16, debug_dumps=False):
    """Build the single-core Bass/Tile program (same program on all 8 cores)."""
    from contextlib import ExitStack

    import concourse.bass as bass
    import concourse.tile as tile
    from concourse import bacc, mybir

    d = _dims(S)
    QB, N_QB, N_KC, R, E_CH, M_CH, O_N = (
        d["QB"], d["N_QB"], d["N_KC"], d["R"], d["E_CH"], d["M_CH"], d["O_N"])

    DT = mybir.dt.bfloat16 if use_bf16 else mybir.dt.float32r
    F32 = mybir.dt.float32
    F32R = mybir.dt.float32r
    AF = mybir.ActivationFunctionType
    ALU = mybir.AluOpType

    nc = bacc.Bacc("TRN2", target_bir_lowering=False, debug=False)

    xq_t = nc.dram_tensor("xq_t", [D, S], DT, kind="ExternalInput").ap()
    xk_t = nc.dram_tensor("xk_t", [D, S], DT, kind="ExternalInput").ap()
    xv_t = nc.dram_tensor("xv_t", [D, S], DT, kind="ExternalInput").ap()
    wq_t = nc.dram_tensor("wq_t", [D, DH], DT, kind="ExternalInput").ap()
    wk_t = nc.dram_tensor("wk_t", [D, DH], DT, kind="ExternalInput").ap()
    wv_t = nc.dram_tensor("wv_t", [D, DH], DT, kind="ExternalInput").ap()
    wo_t = nc.dram_tensor("wo_t", [DH, D], DT, kind="ExternalInput").ap()
    bq_in = nc.dram_tensor("bq_p", [P, M_CH], F32, kind="ExternalInput").ap()
    bk_in = nc.dram_tensor("bk_p", [P, M_CH], F32, kind="ExternalInput").ap()
    bv_in = nc.dram_tensor("bv_r", [P, DH], F32, kind="ExternalInput").ap()
    tri_in = nc.dram_tensor("tri", [P, KC], DT, kind="ExternalInput").ap()
    stepm_in = nc.dram_tensor("stepm", [P, R, QB], DT, kind="ExternalInput").ap()
    ones_c_in = nc.dram_tensor("ones_c", [65, 64], F32R,
                               kind="ExternalInput").ap()
    ones_v_in = nc.dram_tensor("ones_v", [P, N_KC, NH_G, 1], DT,
                               kind="ExternalInput").ap()
    out_p = nc.dram_tensor("out_p", [S, D], F32, kind="ExternalOutput").ap()
    if debug_dumps:
        dbg_qT = nc.dram_tensor("dbg_qT", [P, M_CH, S], DT,
                                kind="ExternalOutput").ap()
        dbg_kT = nc.dram_tensor("dbg_kT", [P, M_CH, S], DT,
                                kind="ExternalOutput").ap()
        dbg_vaug = nc.dram_tensor("dbg_vaug", [P, N_KC, NH_G, 65], DT,
                                  kind="ExternalOutput").ap()
        dbg_probs = nc.dram_tensor("dbg_probs", [P, 2 * QB], DT,
                                   kind="ExternalOutput").ap()
        dbg_attn = nc.dram_tensor("dbg_attn", [M_CH, P, QB], DT,
                                  kind="ExternalOutput").ap()
        dbg_recip = nc.dram_tensor("dbg_recip", [P, 3, QB], F32,
                                   kind="ExternalOutput").ap()

    with tile.TileContext(nc) as tc, ExitStack() as ctx:
        consts = ctx.enter_context(tc.tile_pool(name="consts", bufs=1))
        wpool = ctx.enter_context(tc.tile_pool(name="w", bufs=2))
        qkv = ctx.enter_context(tc.tile_pool(name="qkv", bufs=1))

        tri = consts.tile([P, KC], DT)
        nc.sync.dma_start(tri, tri_in)
        stepm = consts.tile([P, R, QB], DT)
        nc.sync.dma_start(stepm, stepm_in)
        bq_sb = consts.tile([P, M_CH], F32)
        nc.sync.dma_start(bq_sb, bq_in)
        bk_sb = consts.tile([P, M_CH], F32)
        nc.sync.dma_start(bk_sb, bk_in)
        bv_sb = consts.tile([P, DH], F32)
        nc.sync.dma_start(bv_sb, bv_in)
        ones65 = consts.tile([65, 64], F32R)
        nc.sync.dma_start(ones65, ones_c_in)

        qT = qkv.tile([P, M_CH, S], DT, tag="qT")
        kT = qkv.tile([P, M_CH, S], DT, tag="kT")
        v_aug = qkv.tile([P, N_KC, NH_G, 65], DT, tag="v_aug")
        nc.sync.dma_start(v_aug[:, :, :, 64:65], ones_v_in)

        w_tiles = {}
        for name, src in (("wq", wq_t), ("wk", wk_t), ("wv", wv_t)):
            w_sb = wpool.tile([P, E_CH, DH], DT, tag="w")
            nc.sync.dma_start(w_sb, src.rearrange("(eo p) m -> p eo m", p=P))
            w_tiles[name] = w_sb
        wo_sb = wpool.tile([P, M_CH, D], DT, tag="w")
        nc.sync.dma_start(wo_sb, wo_t.rearrange("(mo p) n -> p mo n", p=P))

        # ---- projections ----
        with tc.tile_pool(name="xp", bufs=3) as xpool, \
             tc.tile_pool(name="pj", bufs=3, space="PSUM") as pj_ps:
            for phase, x_in, w_sb, b_sb in (
                ("q", xq_t, w_tiles["wq"], bq_sb),
                ("k", xk_t, w_tiles["wk"], bk_sb),
                ("v", xv_t, w_tiles["wv"], bv_sb),
            ):
                dst = qT if phase == "q" else kT
                for n in range(N_QB):
                    xblk = xpool.tile([P, E_CH, QB], DT, tag="x")
                    nc.sync.dma_start(
                        xblk,
                        x_in.rearrange("(eo p) s -> p eo s", p=P)[
                            :, :, n * QB:(n + 1) * QB],
                    )
                    if phase in ("q", "k"):
                        for m in range(M_CH):
                            ps = pj_ps.tile([P, QB], F32, tag="pj")
                            for e in range(E_CH):
                                nc.tensor.matmul(
                                    ps,
                                    lhsT=w_sb[:, e, m * P:(m + 1) * P],
                                    rhs=xblk[:, e, :],
                                    start=(e == 0), stop=(e == E_CH - 1),
                                )
                            nc.vector.tensor_scalar_add(
                                dst[:, m, n * QB:(n + 1) * QB], ps,
                                b_sb[:, m:m + 1])
                    else:
                        for sc in range(QB // P):
                            ps = pj_ps.tile([P, DH], F32, tag="pj")
                            for e in range(E_CH):
                                nc.tensor.matmul(
                                    ps,
                                    lhsT=xblk[:, e, sc * P:(sc + 1) * P],
                                    rhs=w_sb[:, e, :],
                                    start=(e == 0), stop=(e == E_CH - 1),
                                )
                            kc = n * (QB // P) + sc
                            nc.vector.tensor_tensor(
                                v_aug[:, kc, :, 0:64],
                                ps.rearrange("p (h e) -> p h e", h=NH_G),
                                bv_sb.rearrange("p (h e) -> p h e", h=NH_G),
                                ALU.add,
                            )

        if debug_dumps:
            nc.sync.dma_start(dbg_qT, qT)
            nc.sync.dma_start(dbg_kT, kT)
            nc.sync.dma_start(dbg_vaug, v_aug)

        # ---- attention + output projection ----
        with tc.tile_pool(name="sc_ps", bufs=2, space="PSUM") as sc_ps, \
             tc.tile_pool(name="pv_ps", bufs=2, space="PSUM") as pv_pool, \
             tc.tile_pool(name="rb_ps", bufs=1, space="PSUM") as rb_pool, \
             tc.tile_pool(name="op_ps", bufs=1, space="PSUM") as op_ps, \
             tc.tile_pool(name="probs", bufs=4) as probs_pool, \
             tc.tile_pool(name="attn", bufs=M_CH + 1) as attn_pool, \
             tc.tile_pool(name="misc", bufs=3) as misc, \
             tc.tile_pool(name="outst", bufs=3) as outst:
            for qb in range(N_QB):
                attn_tiles = []
                for m in range(M_CH):
                    n_kc = (qb + 1) * (QB // KC) if causal else N_KC
                    pv_t = [pv_pool.tile([65, QB], F32, tag="pv", name=f"pv{hl}")
                             for hl in (0, 1)]
                    for pair in range(n_kc // 2):
                        ps_h = [sc_ps.tile([P, 2 * QB], F32, tag="sc",
                                          name=f"sc{hl}")
                                for hl in (0, 1)]
                        for dkc in (0, 1):
                            kc = 2 * pair + dkc
                            r = kc - (n_kc - R)
                            is_diag = causal and r >= 0
                            for hl in (0, 1):
                                rows = slice(64 * hl, 64 * hl + 64)
                                nc.tensor.matmul(
                                    ps_h[hl][:, dkc * QB:(dkc + 1) * QB],
                                    lhsT=kT[rows, m, kc * KC:(kc + 1) * KC],
                                    rhs=qT[rows, m, qb * QB:(qb + 1) * QB],
                                    start=True, stop=not is_diag,
                                )
                                if is_diag:
                                    nc.tensor.matmul(
                                        ps_h[hl][:, dkc * QB:(dkc + 1) * QB],
                                        lhsT=tri, rhs=stepm[:, r, :],
                                        start=False, stop=True,
                                    )
                        for hl in (0, 1):
                            pt = probs_pool.tile([P, 2 * QB], DT, tag="pt")
                            nc.scalar.activation(pt, ps_h[hl], AF.Exp,
                                                 scale=float(SCALE))
                            if (debug_dumps and qb == 0 and m == 0
                                    and pair == 0 and hl == 0):
                                nc.sync.dma_start(dbg_probs, pt)
                            for dkc in (0, 1):
                                kc = 2 * pair + dkc
                                nc.tensor.matmul(
                                    pv_t[hl],
                                    lhsT=v_aug[:, kc, 2 * m + hl, :],
                                    rhs=pt[:, dkc * QB:(dkc + 1) * QB],
                                    start=(kc == 0), stop=(kc == n_kc - 1),
                                )
                    attn_m = attn_pool.tile([P, QB], DT, tag="attn")
                    for hl in (0, 1):
                        # Drain PSUM fast (attn + sums copies) so the PV tile
                        # recycles without waiting on the slow reciprocal.
                        attn_u = misc.tile([64, QB], DT, tag="attn_u")
                        nc.any.tensor_copy(attn_u, pv_t[hl][0:64, :])
                        # 1/sums as exp(-ln(sums)) on ACT: same table set as
                        # the softmax exp, ~4x faster than DVE reciprocal and
                        # off the DVE.
                        ltmp = misc.tile([65, QB], F32, tag="ltmp")
                        nc.scalar.activation(ltmp[64:65, :],
                                             pv_t[hl][64:65, :], AF.Ln)
                        recip65 = misc.tile([65, QB], F32R, tag="recip")
                        nc.scalar.activation(recip65[64:65, :],
                                             ltmp[64:65, :], AF.Exp,
                                             scale=-1.0)
                        rb = rb_pool.tile([64, QB], F32, tag="rb")
                        nc.tensor.matmul(rb, lhsT=ones65[64:65, :],
                                         rhs=recip65[64:65, :],
                                         start=True, stop=True)
                        if debug_dumps and qb == 0 and m == 0 and hl == 0:
                            nc.sync.dma_start(
                                dbg_recip[64:65, 1, :],
                                recip65[64:65, :].bitcast(F32))
                            nc.sync.dma_start(
                                dbg_recip[64:65, 0, :],
                                recip65[64:65, :].bitcast(F32))
                            rbsb = misc.tile([64, QB], F32, tag="rbsb",
                                             name="rbsb")
                            nc.vector.tensor_copy(rbsb, rb)
                            nc.sync.dma_start(dbg_recip[0:64, 2, :], rbsb)
                        nc.vector.tensor_tensor(
                            attn_m[64 * hl:64 * hl + 64, :], attn_u, rb,
                            ALU.mult)
                    if debug_dumps and qb == 0:
                        nc.sync.dma_start(dbg_attn[m], attn_m)
                    attn_tiles.append(attn_m)
                for ssub in range(QB // P):
                    for nout in range(O_N):
                        pso = op_ps.tile([P, 512], F32, tag="op")
                        for m in range(M_CH):
                            nc.tensor.matmul(
                                pso,
                                lhsT=attn_tiles[m][:, ssub * P:(ssub + 1) * P],
                                rhs=wo_sb[:, m, nout * 512:(nout + 1) * 512],
                                start=(m == 0), stop=(m == M_CH - 1),
                            )
                        st = outst.tile([P, 512], F32, tag="st")
                        nc.any.tensor_copy(st, pso)
                        nc.sync.dma_start(
                            out_p[qb * QB + ssub * P: qb * QB + (ssub + 1) * P,
                                  nout * 512:(nout + 1) * 512],
                            st)
    nc.compile()
    return nc


def make_consts(S, use_bf16):
    """Host-built mask-bias matmul operands (tri, stepm)."""
    d = _dims(S)
    QB, R = d["QB"], d["R"]
    npdt = _np_dt(use_bf16)
    tri = np.zeros((P, KC), np.float32)
    for t in range(P):
        tri[t, t:] = 1.0
    stepm = np.zeros((P, R, QB), np.float32)
    for r in range(R):
        for j in range(QB):
            c = j - KC * r
            if c >= KC - 1:
                continue
            stepm[max(0, c + 1), r, j] = NEG
    return tri.astype(npdt), stepm.astype(npdt)


def core_inputs(Q, K, V, W_q, b_q, W_k, b_k, W_v, b_v, W_o, b, hg, S, use_bf16):
    """Build the per-core input map (host-side slicing/transposition/casts)."""
    npdt = _np_dt(use_bf16)
    d = _dims(S)
    M_CH = d["M_CH"]
    rows = slice(hg * DH, (hg + 1) * DH)

    def t(x):
        return np.ascontiguousarray(np.asarray(x, np.float32).T).astype(npdt)

    tri, stepm = make_consts(S, use_bf16)
    return {
        "xq_t": t(Q[b]), "xk_t": t(K[b]), "xv_t": t(V[b]),
        "wq_t": t(W_q[rows]), "wk_t": t(W_k[rows]), "wv_t": t(W_v[rows]),
        "wo_t": t(W_o[:, rows]),
        "bq_p": np.ascontiguousarray(
            np.asarray(b_q[rows], np.float32).reshape(M_CH, P).T),
        "bk_p": np.ascontiguousarray(
            np.asarray(b_k[rows], np.float32).reshape(M_CH, P).T),
        "bv_r": np.broadcast_to(
            np.asarray(b_v[rows], np.float32), (P, DH)).copy(),
        "tri": tri, "stepm": stepm,
        "ones_c": np.ones((65, 64), np.float32),
        "ones_v": np.ones((P, d["N_KC"], NH_G, 1), npdt),
    }


def _np_reference(Q, K, V, mask, W_q, b_q, W_k, b_k, W_v, b_v, W_o, b_o):
    """Exact numpy fallback for arbitrary masks."""
    q = (Q @ W_q.T + b_q).reshape(B, S_FULL, H, DK).transpose(0, 2, 1, 3)
    k = (K @ W_k.T + b_k).reshape(B, S_FULL, H, DK).transpose(0, 2, 1, 3)
    v = (V @ W_v.T + b_v).reshape(B, S_FULL, H, DK).transpose(0, 2, 1, 3)
    scores = np.einsum("bhqd,bhkd->bhqk", q, k) / np.sqrt(np.float32(DK))
    scores = np.where(mask == 0, np.finfo(np.float32).min, scores)
    scores -= scores.max(-1, keepdims=True)
    probs = np.exp(scores)
    probs /= probs.sum(-1, keepdims=True)
    out = np.einsum("bhqk,bhkd->bhqd", probs, v)
    out = out.transpose(0, 2, 1, 3).reshape(B, S_FULL, D)
    return (out @ W_o.T + b_o).astype(np.float32)


def kernel(Q, K, V, mask, W_q, b_q, W_k, b_k, W_v, b_v, W_o, b_o):
    Q = np.asarray(Q, np.float32)
    K = np.asarray(K, np.float32)
    V = np.asarray(V, np.float32)
    mask = np.asarray(mask)

    m2 = mask.reshape(mask.shape[-2], mask.shape[-1])
    if np.array_equal(m2 != 0, np.tril(np.ones(m2.shape, bool))):
        causal = True
    elif (m2 != 0).all():
        causal = False
    else:
        return _np_reference(Q, K, V, mask, W_q, b_q, W_k, b_k, W_v, b_v,
                             W_o, b_o)

    use_bf16 = os.environ.get("MHA_KERNEL_DTYPE", "f32r") == "bf16"
    import concourse.bass_utils as _bu
    from concourse.bass_utils import run_bass_kernel_spmd
    if (os.environ.get("MHA_LDW_OPT", "0") == "1"
            and not getattr(_bu, "_mha_ldw_patched", False)):
        _orig_rc = _bu.run_command

        def _rc(argv, **kw):
            argv = ["--enable-ldw-opt=true" if a == "--enable-ldw-opt=false"
                    else a for a in argv]
            return _orig_rc(argv, **kw)

        _bu.run_command = _rc
        _bu._mha_ldw_patched = True

    key = (causal, S_FULL, use_bf16)
    if key not in _PROG_CACHE:
        _PROG_CACHE[key] = build_program(causal, S_FULL, use_bf16)
    nc = _PROG_CACHE[key]

    in_maps = []
    for c in range(8):
        b, hg = divmod(c, 2)
        in_maps.append(core_inputs(Q, K, V, W_q, b_q, W_k, b_k, W_v, b_v,
                                   W_o, b, hg, S_FULL, use_bf16))

    trace = os.environ.get("MHA_KERNEL_TRACE", "0") == "1"
    kw = {}
    if trace:
        kw = {"trace": True,
              "trace_cores": [int(x) for x in os.environ.get(
                  "MHA_TRACE_CORES", "0").split(",")]}
    res = run_bass_kernel_spmd(nc, in_maps, core_ids=list(range(8)), **kw)
    kernel.last_results = res

    b_o32 = np.asarray(b_o, np.float32)
    out = np.empty((B, S_FULL, D), np.float32)
    for b in range(B):
        out[b] = (res.results[2 * b]["out_p"] + res.results[2 * b + 1]["out_p"]
                  + b_o32[None, :])
    return out


kernel.last_results = None


# revision 18
# speedup vs baseline: 1.3675x; 1.1879x over previous
"""MultiHeadAttention Trainium2 kernel (8 NeuronCores).

Sharding: core c handles batch b = c // 2 and head-group hg = c % 2
(8 of 16 heads, 512 of 1024 model dims). Attention is embarrassingly
parallel over (b, hg); the output projection is computed per head-group
against the matching W_o columns, yielding partial outputs that the host
sums (plus b_o).

Device dataflow (per core), all in "transposed" layouts so no on-device
transposes are ever needed:
  qT = Wq_hg @ Xq^T      [dh=512, S]   (lhsT = Wq_hg^T, rhs = Xq^T; host preps both)
  kT = Wk_hg @ Xk^T      [dh=512, S]
  v  = Xv @ Wv_hg^T      [S, dh=512]   (+ ones column per head for softmax sums)
  scores_T[k, q] = kT_h[:, kchunk]^T-matmul  (keys on partitions)
  causal mask: extra PE matmul tri^T @ step accumulating -1e9 into masked entries
  probs = exp(scores_T / 8) on ACT (no max subtraction: scores ~ N(0,1), safe)
  attn_T[d, q] (+ sums row) = v_chunk^T-matmul over probs, accumulated in PSUM
  normalize: recip = 1/sums (DVE), broadcast via ones-matmul, multiply (DVE)
  out_partial = attn^T-matmul with Wo columns
"""

import os

import numpy as np

B, S_FULL, D = 4, 2048, 1024
H, DK = 16, 64
NH_G = 8          # heads per core
DH = NH_G * DK    # 512 dims per core
P = 128
KC = 128          # key chunk (PE contraction)
NEG = -1.0e9
SCALE = 1.0 / np.sqrt(np.float32(DK))

_PROG_CACHE = {}


def _dims(S):
    QB = min(512, S)
    return {
        "S": S, "QB": QB, "N_QB": S // QB, "N_KC": S // KC,
        "R": QB // KC, "E_CH": D // P, "M_CH": DH // P, "O_N": D // 512,
    }


def _np_dt(use_bf16):
    if use_bf16:
        import ml_dtypes
        return ml_dtypes.bfloat16
    return np.float32


def build_program(causal, S, use_bf16, debug_dumps=False):
    """Build the single-core Bass/Tile program (same program on all 8 cores)."""
    from contextlib import ExitStack

    import concourse.bass as bass
    import concourse.tile as tile
    from concourse import bacc, mybir

    d = _dims(S)
    QB, N_QB, N_KC, R, E_CH, M_CH, O_N = (
        d["QB"], d["N_QB"], d["N_KC"], d["R"], d["E_CH"], d["M_CH"], d["O_N"])

    DT = mybir.dt.bfloat16 if use_bf16 else mybir.dt.float32r
    F32 = mybir.dt.float32
    F32R = mybir.dt.float32r
    AF = mybir.ActivationFunctionType
    ALU = mybir.AluOpType

    nc = bacc.Bacc("TRN2", target_bir_lowering=False, debug=False)

    xq_t = nc.dram_tensor("xq_t", [D, S], DT, kind="ExternalInput").ap()
    xk_t = nc.dram_tensor("xk_t", [D, S], DT, kind="ExternalInput").ap()
    xv_t = nc.dram_tensor("xv_t", [D, S], DT, kind="ExternalInput").ap()
    wq_t = nc.dram_tensor("wq_t", [D, DH], DT, kind="ExternalInput").ap()
    wk_t = nc.dram_tensor("wk_t", [D, DH], DT, kind="ExternalInput").ap()
    wv_t = nc.dram_tensor("wv_t", [D, DH], DT, kind="ExternalInput").ap()
    wo_t = nc.dram_tensor("wo_t", [DH, D], DT, kind="ExternalInput").ap()
    bq_in = nc.dram_tensor("bq_p", [P, M_CH], F32, kind="ExternalInput").ap()
    bk_in = nc.dram_tensor("bk_p", [P, M_CH], F32, kind="ExternalInput").ap()
    bv_in = nc.dram_tensor("bv_r", [P, DH], F32, kind="ExternalInput").ap()
    dmask_in = nc.dram_tensor("dmask", [P, R, QB], DT,
                              kind="ExternalInput").ap()
    ones_c_in = nc.dram_tensor("ones_c", [65, 64], F32R,
                               kind="ExternalInput").ap()
    ones_v_in = nc.dram_tensor("ones_v", [P, N_KC, NH_G, 1], DT,
                               kind="ExternalInput").ap()
    out_p = nc.dram_tensor("out_p", [S, D], F32, kind="ExternalOutput").ap()
    if debug_dumps:
        dbg_qT = nc.dram_tensor("dbg_qT", [P, M_CH, S], DT,
                                kind="ExternalOutput").ap()
        dbg_kT = nc.dram_tensor("dbg_kT", [P, M_CH, S], DT,
                                kind="ExternalOutput").ap()
        dbg_vaug = nc.dram_tensor("dbg_vaug", [P, N_KC, NH_G, 65], DT,
                                  kind="ExternalOutput").ap()
        dbg_probs = nc.dram_tensor("dbg_probs", [P, QB], DT,
                                   kind="ExternalOutput").ap()
        dbg_attn = nc.dram_tensor("dbg_attn", [M_CH, P, QB], DT,
                                  kind="ExternalOutput").ap()
        dbg_recip = nc.dram_tensor("dbg_recip", [P, 3, QB], F32,
                                   kind="ExternalOutput").ap()

    with tile.TileContext(nc) as tc, ExitStack() as ctx:
        consts = ctx.enter_context(tc.tile_pool(name="consts", bufs=1))
        wpool = ctx.enter_context(tc.tile_pool(name="w", bufs=2))
        qkv = ctx.enter_context(tc.tile_pool(name="qkv", bufs=1))

        dmask = consts.tile([P, R, QB], DT)
        nc.sync.dma_start(dmask, dmask_in)
        bq_sb = consts.tile([P, M_CH], F32)
        nc.sync.dma_start(bq_sb, bq_in)
        bk_sb = consts.tile([P, M_CH], F32)
        nc.sync.dma_start(bk_sb, bk_in)
        bv_sb = consts.tile([P, DH], F32)
        nc.sync.dma_start(bv_sb, bv_in)
        ones65 = consts.tile([65, 64], F32R)
        nc.sync.dma_start(ones65, ones_c_in)

        qT = qkv.tile([P, M_CH, S], DT, tag="qT")
        kT = qkv.tile([P, M_CH, S], DT, tag="kT")
        v_aug = qkv.tile([P, N_KC, NH_G, 65], DT, tag="v_aug")
        nc.sync.dma_start(v_aug[:, :, :, 64:65], ones_v_in)

        w_tiles = {}
        for name, src in (("wq", wq_t), ("wk", wk_t), ("wv", wv_t)):
            w_sb = wpool.tile([P, E_CH, DH], DT, tag="w")
            wr = src.rearrange("(eo p) m -> p eo m", p=P)
            for e in range(E_CH):
                nc.sync.dma_start(w_sb[:, e], wr[:, e])
            w_tiles[name] = w_sb
        wo_sb = wpool.tile([P, M_CH, D], DT, tag="w")
        nc.sync.dma_start(wo_sb, wo_t.rearrange("(mo p) n -> p mo n", p=P))

        # ---- projections ----
        with tc.tile_pool(name="xp", bufs=3) as xpool, \
             tc.tile_pool(name="pj", bufs=3, space="PSUM") as pj_ps:
            for phase, x_in, w_sb, b_sb in (
                ("q", xq_t, w_tiles["wq"], bq_sb),
                ("k", xk_t, w_tiles["wk"], bk_sb),
                ("v", xv_t, w_tiles["wv"], bv_sb),
            ):
                dst = qT if phase == "q" else kT
                for n in range(N_QB):
                    xblk = xpool.tile([P, E_CH, QB], DT, tag="x")
                    xr = x_in.rearrange("(eo p) s -> p eo s", p=P)
                    for e in range(E_CH):
                        nc.sync.dma_start(
                            xblk[:, e], xr[:, e, n * QB:(n + 1) * QB])
                    if phase in ("q", "k"):
                        for m in range(M_CH):
                            ps = pj_ps.tile([P, QB], F32, tag="pj")
                            for e in range(E_CH):
                                nc.tensor.matmul(
                                    ps,
                                    lhsT=w_sb[:, e, m * P:(m + 1) * P],
                                    rhs=xblk[:, e, :],
                                    start=(e == 0), stop=(e == E_CH - 1),
                                )
                            nc.vector.tensor_scalar_add(
                                dst[:, m, n * QB:(n + 1) * QB], ps,
                                b_sb[:, m:m + 1])
                    else:
                        for sc in range(QB // P):
                            ps = pj_ps.tile([P, DH], F32, tag="pj")
                            for e in range(E_CH):
                                nc.tensor.matmul(
                                    ps,
                                    lhsT=xblk[:, e, sc * P:(sc + 1) * P],
                                    rhs=w_sb[:, e, :],
                                    start=(e == 0), stop=(e == E_CH - 1),
                                )
                            kc = n * (QB // P) + sc
                            nc.vector.tensor_tensor(
                                v_aug[:, kc, :, 0:64],
                                ps.rearrange("p (h e) -> p h e", h=NH_G),
                                bv_sb.rearrange("p (h e) -> p h e", h=NH_G),
                                ALU.add,
                            )

        if debug_dumps:
            nc.sync.dma_start(dbg_qT, qT)
            nc.sync.dma_start(dbg_kT, kT)
            nc.sync.dma_start(dbg_vaug, v_aug)

        # ---- attention + output projection ----
        with tc.tile_pool(name="sc_ps", bufs=4, space="PSUM") as sc_ps, \
             tc.tile_pool(name="pv_ps", bufs=2, space="PSUM") as pv_pool, \
             tc.tile_pool(name="rb_ps", bufs=1, space="PSUM") as rb_pool, \
             tc.tile_pool(name="op_ps", bufs=1, space="PSUM") as op_ps, \
             tc.tile_pool(name="probs", bufs=6) as probs_pool, \
             tc.tile_pool(name="attn", bufs=M_CH + 1) as attn_pool, \
             tc.tile_pool(name="misc", bufs=3) as misc, \
             tc.tile_pool(name="outst", bufs=3) as outst:
            for qb in range(N_QB):
                attn_tiles = []
                for m in range(M_CH):
                    n_kc = (qb + 1) * (QB // KC) if causal else N_KC
                    pv_t = [pv_pool.tile([65, QB], F32, tag="pv", name=f"pv{hl}")
                             for hl in (0, 1)]
                    for kc in range(n_kc):
                        r = kc - (n_kc - R)
                        is_diag = causal and r >= 0
                        ps_h = [sc_ps.tile([P, QB], F32, tag="sc",
                                          name=f"sc{hl}")
                                for hl in (0, 1)]
                        for hl in (0, 1):
                            rows = slice(64 * hl, 64 * hl + 64)
                            nc.tensor.matmul(
                                ps_h[hl],
                                lhsT=kT[rows, m, kc * KC:(kc + 1) * KC],
                                rhs=qT[rows, m, qb * QB:(qb + 1) * QB],
                                start=True, stop=True,
                            )
                        for hl in (0, 1):
                            pt = probs_pool.tile([P, QB], DT, tag="pt")
                            nc.scalar.activation(pt, ps_h[hl], AF.Exp,
                                                 scale=float(SCALE))
                            if is_diag:
                                nc.vector.tensor_tensor(
                                    pt, pt, dmask[:, r, :], ALU.mult)
                            if (debug_dumps and qb == 0 and m == 0
                                    and kc == 0 and hl == 0):
                                nc.sync.dma_start(dbg_probs, pt)
                            nc.tensor.matmul(
                                pv_t[hl],
                                lhsT=v_aug[:, kc, 2 * m + hl, :],
                                rhs=pt,
                                start=(kc == 0), stop=(kc == n_kc - 1),
                            )
                    attn_m = attn_pool.tile([P, QB], DT, tag="attn")
                    for hl in (0, 1):
                        # Drain PSUM fast (attn + sums copies) so the PV tile
                        # recycles without waiting on the slow reciprocal.
                        attn_u = misc.tile([64, QB], DT, tag="attn_u")
                        nc.any.tensor_copy(attn_u, pv_t[hl][0:64, :])
                        recip65 = misc.tile([65, QB], F32R, tag="recip")
                        with nc.allow_low_precision(
                                reason="softmax denom recip, f32r rounding"):
                            nc.vector.reciprocal(recip65[64:65, :],
                                                 pv_t[hl][64:65, :])
                        rb = rb_pool.tile([64, QB], F32, tag="rb")
                        nc.tensor.matmul(rb, lhsT=ones65[64:65, :],
                                         rhs=recip65[64:65, :],
                                         start=True, stop=True)
                        if debug_dumps and qb == 0 and m == 0 and hl == 0:
                            nc.sync.dma_start(
                                dbg_recip[64:65, 1, :],
                                recip65[64:65, :].bitcast(F32))
                            nc.sync.dma_start(
                                dbg_recip[64:65, 0, :],
                                recip65[64:65, :].bitcast(F32))
                            rbsb = misc.tile([64, QB], F32, tag="rbsb",
                                             name="rbsb")
                            nc.vector.tensor_copy(rbsb, rb)
                            nc.sync.dma_start(dbg_recip[0:64, 2, :], rbsb)
                        nc.vector.tensor_tensor(
                            attn_m[64 * hl:64 * hl + 64, :], attn_u, rb,
                            ALU.mult)
                    if debug_dumps and qb == 0:
                        nc.sync.dma_start(dbg_attn[m], attn_m)
                    attn_tiles.append(attn_m)
                for ssub in range(QB // P):
                    for nout in range(O_N):
                        pso = op_ps.tile([P, 512], F32, tag="op")
                        for m in range(M_CH):
                            nc.tensor.matmul(
                                pso,
                                lhsT=attn_tiles[m][:, ssub * P:(ssub + 1) * P],
                                rhs=wo_sb[:, m, nout * 512:(nout + 1) * 512],
                                start=(m == 0), stop=(m == M_CH - 1),
                            )
                        st = outst.tile([P, 512], F32, tag="st")
                        nc.any.tensor_copy(st, pso)
                        nc.sync.dma_start(
                            out_p[qb * QB + ssub * P: qb * QB + (ssub + 1) * P,
                                  nout * 512:(nout + 1) * 512],
                            st)
    nc.compile()
    return nc


def make_consts(S, use_bf16):
    """Host-built 0/1 causal masks for the R diagonal key-chunks."""
    d = _dims(S)
    QB, R = d["QB"], d["R"]
    npdt = _np_dt(use_bf16)
    i = np.arange(P)[:, None]
    j = np.arange(QB)[None, :]
    dmask = np.stack([(i <= j - KC * r) for r in range(R)], 1)
    return dmask.astype(npdt)


def core_inputs(Q, K, V, W_q, b_q, W_k, b_k, W_v, b_v, W_o, b, hg, S, use_bf16):
    """Build the per-core input map (host-side slicing/transposition/casts)."""
    npdt = _np_dt(use_bf16)
    d = _dims(S)
    M_CH = d["M_CH"]
    rows = slice(hg * DH, (hg + 1) * DH)

    def t(x):
        return np.ascontiguousarray(np.asarray(x, np.float32).T).astype(npdt)

    dmask = make_consts(S, use_bf16)
    return {
        "xq_t": t(Q[b]), "xk_t": t(K[b]), "xv_t": t(V[b]),
        "wq_t": t(W_q[rows]), "wk_t": t(W_k[rows]), "wv_t": t(W_v[rows]),
        "wo_t": t(W_o[:, rows]),
        "bq_p": np.ascontiguousarray(
            np.asarray(b_q[rows], np.float32).reshape(M_CH, P).T),
        "bk_p": np.ascontiguousarray(
            np.asarray(b_k[rows], np.float32).reshape(M_CH, P).T),
        "bv_r": np.broadcast_to(
            np.asarray(b_v[rows], np.float32), (P, DH)).copy(),
        "dmask": dmask,
        "ones_c": np.ones((65, 64), np.float32),
        "ones_v": np.ones((P, d["N_KC"], NH_G, 1), npdt),
    }


def _np_reference(Q, K, V, mask, W_q, b_q, W_k, b_k, W_v, b_v, W_o, b_o):
    """Exact numpy fallback for arbitrary masks."""
    q = (Q @ W_q.T + b_q).reshape(B, S_FULL, H, DK).transpose(0, 2, 1, 3)
    k = (K @ W_k.T + b_k).reshape(B, S_FULL, H, DK).transpose(0, 2, 1, 3)
    v = (V @ W_v.T + b_v).reshape(B, S_FULL, H, DK).transpose(0, 2, 1, 3)
    scores = np.einsum("bhqd,bhkd->bhqk", q, k) / np.sqrt(np.float32(DK))
    scores = np.where(mask == 0, np.finfo(np.float32).min, scores)
    scores -= scores.max(-1, keepdims=True)
    probs = np.exp(scores)
    probs /= probs.sum(-1, keepdims=True)
    out = np.einsum("bhqk,bhkd->bhqd", probs, v)
    out = out.transpose(0, 2, 1, 3).reshape(B, S_FULL, D)
    return (out @ W_o.T + b_o).astype(np.float32)


def kernel(Q, K, V, mask, W_q, b_q, W_k, b_k, W_v, b_v, W_o, b_o):
    Q = np.asarray(Q, np.float32)
    K = np.asarray(K, np.float32)
    V = np.asarray(V, np.float32)
    mask = np.asarray(mask)

    m2 = mask.reshape(mask.shape[-2], mask.shape[-1])
    if np.array_equal(m2 != 0, np.tril(np.ones(m2.shape, bool))):
        causal = True
    elif (m2 != 0).all():
        causal = False
    else:
        return _np_reference(Q, K, V, mask, W_q, b_q, W_k, b_k, W_v, b_v,
                             W_o, b_o)

    use_bf16 = os.environ.get("MHA_KERNEL_DTYPE", "f32r") == "bf16"
    import concourse.bass_utils as _bu
    from concourse.bass_utils import run_bass_kernel_spmd
    if (os.environ.get("MHA_LDW_OPT", "0") == "1"
            and not getattr(_bu, "_mha_ldw_patched", False)):
        _orig_rc = _bu.run_command

        def _rc(argv, **kw):
            argv = ["--enable-ldw-opt=true" if a == "--enable-ldw-opt=false"
                    else a for a in argv]
            return _orig_rc(argv, **kw)

        _bu.run_command = _rc
        _bu._mha_ldw_patched = True

    key = (causal, S_FULL, use_bf16)
    if key not in _PROG_CACHE:
        _PROG_CACHE[key] = build_program(causal, S_FULL, use_bf16)
    nc = _PROG_CACHE[key]

    in_maps = []
    for c in range(8):
        b, hg = divmod(c, 2)
        in_maps.append(core_inputs(Q, K, V, W_q, b_q, W_k, b_k, W_v, b_v,
                                   W_o, b, hg, S_FULL, use_bf16))

    trace = os.environ.get("MHA_KERNEL_TRACE", "0") == "1"
    kw = {}
    if trace:
        kw = {"trace": True,
              "trace_cores": [int(x) for x in os.environ.get(
                  "MHA_TRACE_CORES", "0").split(",")]}
    n_cores = int(os.environ.get("MHA_CORES", "8"))
    res = run_bass_kernel_spmd(nc, in_maps[:n_cores],
                               core_ids=list(range(n_cores)), **kw)
    kernel.last_results = res

    b_o32 = np.asarray(b_o, np.float32)
    out = np.zeros((B, S_FULL, D), np.float32)
    for b in range(B):
        if 2 * b + 1 < n_cores:
            out[b] = (res.results[2 * b]["out_p"]
                      + res.results[2 * b + 1]["out_p"] + b_o32[None, :])
    return out


kernel.last_results = None


# revision 20
# speedup vs baseline: 1.6803x; 1.2287x over previous
"""MultiHeadAttention Trainium2 kernel (8 NeuronCores).

Sharding: core c handles batch b = c // 2 and head-group hg = c % 2
(8 of 16 heads, 512 of 1024 model dims). Attention is embarrassingly
parallel over (b, hg); the output projection is computed per head-group
against the matching W_o columns, yielding partial outputs that the host
sums (plus b_o).

Device dataflow (per core), all in "transposed" layouts so no on-device
transposes are ever needed:
  qT = Wq_hg @ Xq^T      [dh=512, S]   (lhsT = Wq_hg^T, rhs = Xq^T; host preps both)
  kT = Wk_hg @ Xk^T      [dh=512, S]
  v  = Xv @ Wv_hg^T      [S, dh=512]   (+ ones column per head for softmax sums)
  scores_T[k, q] = kT_h[:, kchunk]^T-matmul  (keys on partitions)
  causal mask: extra PE matmul tri^T @ step accumulating -1e9 into masked entries
  probs = exp(scores_T / 8) on ACT (no max subtraction: scores ~ N(0,1), safe)
  attn_T[d, q] (+ sums row) = v_chunk^T-matmul over probs, accumulated in PSUM
  normalize: recip = 1/sums (DVE), broadcast via ones-matmul, multiply (DVE)
  out_partial = attn^T-matmul with Wo columns
"""

import os

import numpy as np

B, S_FULL, D = 4, 2048, 1024
H, DK = 16, 64
NH_G = 8          # heads per core
DH = NH_G * DK    # 512 dims per core
P = 128
KC = 128          # key chunk (PE contraction)
NEG = -1.0e9
SCALE = 1.0 / np.sqrt(np.float32(DK))

_PROG_CACHE = {}


def _dims(S):
    QB = min(512, S)
    return {
        "S": S, "QB": QB, "N_QB": S // QB, "N_KC": S // KC,
        "R": QB // KC, "E_CH": D // P, "M_CH": DH // P, "O_N": D // 512,
    }


def _np_dt(use_bf16):
    if use_bf16:
        import ml_dtypes
        return ml_dtypes.bfloat16
    return np.float32


def build_program(causal, S, use_bf16, debug_dumps=False):
    """Build the single-core Bass/Tile program (same program on all 8 cores)."""
    from contextlib import ExitStack

    import concourse.bass as bass
    import concourse.tile as tile
    from concourse import bacc, mybir

    d = _dims(S)
    QB, N_QB, N_KC, R, E_CH, M_CH, O_N = (
        d["QB"], d["N_QB"], d["N_KC"], d["R"], d["E_CH"], d["M_CH"], d["O_N"])

    DT = mybir.dt.bfloat16 if use_bf16 else mybir.dt.float32r
    F32 = mybir.dt.float32
    F32R = mybir.dt.float32r
    AF = mybir.ActivationFunctionType
    ALU = mybir.AluOpType

    nc = bacc.Bacc("TRN2", target_bir_lowering=False, debug=False)

    xq_t = nc.dram_tensor("xq_t", [D, S], DT, kind="ExternalInput").ap()
    xk_t = nc.dram_tensor("xk_t", [D, S], DT, kind="ExternalInput").ap()
    xv_t = nc.dram_tensor("xv_t", [D, S], DT, kind="ExternalInput").ap()
    wq_t = nc.dram_tensor("wq_t", [D, DH], DT, kind="ExternalInput").ap()
    wk_t = nc.dram_tensor("wk_t", [D, DH], DT, kind="ExternalInput").ap()
    wv_t = nc.dram_tensor("wv_t", [D, DH], DT, kind="ExternalInput").ap()
    wo_t = nc.dram_tensor("wo_t", [DH, D], DT, kind="ExternalInput").ap()
    bq_in = nc.dram_tensor("bq_p", [P, M_CH], F32, kind="ExternalInput").ap()
    bk_in = nc.dram_tensor("bk_p", [P, M_CH], F32, kind="ExternalInput").ap()
    bv_in = nc.dram_tensor("bv_r", [P, DH], F32, kind="ExternalInput").ap()
    dmask_in = nc.dram_tensor("dmask", [P, R, QB], DT,
                              kind="ExternalInput").ap()
    ones_c_in = nc.dram_tensor("ones_c", [65, 64], F32R,
                               kind="ExternalInput").ap()
    ones_v_in = nc.dram_tensor("ones_v", [P, N_KC, NH_G, 1], DT,
                               kind="ExternalInput").ap()
    out_p = nc.dram_tensor("out_p", [S, D], F32, kind="ExternalOutput").ap()
    if debug_dumps:
        dbg_qT = nc.dram_tensor("dbg_qT", [P, M_CH, S], DT,
                                kind="ExternalOutput").ap()
        dbg_kT = nc.dram_tensor("dbg_kT", [P, M_CH, S], DT,
                                kind="ExternalOutput").ap()
        dbg_vaug = nc.dram_tensor("dbg_vaug", [P, N_KC, NH_G, 65], DT,
                                  kind="ExternalOutput").ap()
        dbg_probs = nc.dram_tensor("dbg_probs", [P, QB], DT,
                                   kind="ExternalOutput").ap()
        dbg_attn = nc.dram_tensor("dbg_attn", [M_CH, P, QB], DT,
                                  kind="ExternalOutput").ap()
        dbg_recip = nc.dram_tensor("dbg_recip", [P, 3, QB], F32,
                                   kind="ExternalOutput").ap()

    with tile.TileContext(nc) as tc, ExitStack() as ctx:
        consts = ctx.enter_context(tc.tile_pool(name="consts", bufs=1))
        wpool = ctx.enter_context(tc.tile_pool(name="w", bufs=2))
        qkv = ctx.enter_context(tc.tile_pool(name="qkv", bufs=1))

        dmask = consts.tile([P, R, QB], DT)
        nc.sync.dma_start(dmask, dmask_in)
        bq_sb = consts.tile([P, M_CH], F32)
        nc.sync.dma_start(bq_sb, bq_in)
        bk_sb = consts.tile([P, M_CH], F32)
        nc.sync.dma_start(bk_sb, bk_in)
        bv_sb = consts.tile([P, DH], F32)
        nc.sync.dma_start(bv_sb, bv_in)
        ones65 = consts.tile([65, 64], F32R)
        nc.sync.dma_start(ones65, ones_c_in)

        qT = qkv.tile([P, M_CH, S], DT, tag="qT")
        kT = qkv.tile([P, M_CH, S], DT, tag="kT")
        v_aug = qkv.tile([P, N_KC, NH_G, 65], DT, tag="v_aug")
        nc.sync.dma_start(v_aug[:, :, :, 64:65], ones_v_in)

        w_tiles = {}
        for name, src in (("wq", wq_t), ("wk", wk_t), ("wv", wv_t)):
            w_sb = wpool.tile([P, E_CH, DH], DT, tag="w")
            wr = src.rearrange("(eo p) m -> p eo m", p=P)
            for e in range(E_CH):
                nc.sync.dma_start(w_sb[:, e], wr[:, e])
            w_tiles[name] = w_sb
        wo_sb = wpool.tile([P, M_CH, D], DT, tag="w")
        nc.sync.dma_start(wo_sb, wo_t.rearrange("(mo p) n -> p mo n", p=P))

        # ---- projections ----
        with tc.tile_pool(name="xp", bufs=3) as xpool, \
             tc.tile_pool(name="pj", bufs=3, space="PSUM") as pj_ps:
            for phase, x_in, w_sb, b_sb in (
                ("q", xq_t, w_tiles["wq"], bq_sb),
                ("k", xk_t, w_tiles["wk"], bk_sb),
                ("v", xv_t, w_tiles["wv"], bv_sb),
            ):
                dst = qT if phase == "q" else kT
                for n in range(N_QB):
                    xblk = xpool.tile([P, E_CH, QB], DT, tag="x")
                    xr = x_in.rearrange("(eo p) s -> p eo s", p=P)
                    for e in range(E_CH):
                        nc.sync.dma_start(
                            xblk[:, e], xr[:, e, n * QB:(n + 1) * QB])
                    if phase in ("q", "k"):
                        for m in range(M_CH):
                            ps = pj_ps.tile([P, QB], F32, tag="pj")
                            for e in range(E_CH):
                                nc.tensor.matmul(
                                    ps,
                                    lhsT=w_sb[:, e, m * P:(m + 1) * P],
                                    rhs=xblk[:, e, :],
                                    start=(e == 0), stop=(e == E_CH - 1),
                                )
                            nc.vector.tensor_scalar_add(
                                dst[:, m, n * QB:(n + 1) * QB], ps,
                                b_sb[:, m:m + 1])
                    else:
                        for sc in range(QB // P):
                            ps = pj_ps.tile([P, DH], F32, tag="pj")
                            for e in range(E_CH):
                                nc.tensor.matmul(
                                    ps,
                                    lhsT=xblk[:, e, sc * P:(sc + 1) * P],
                                    rhs=w_sb[:, e, :],
                                    start=(e == 0), stop=(e == E_CH - 1),
                                )
                            kc = n * (QB // P) + sc
                            nc.vector.tensor_tensor(
                                v_aug[:, kc, :, 0:64],
                                ps.rearrange("p (h e) -> p h e", h=NH_G),
                                bv_sb.rearrange("p (h e) -> p h e", h=NH_G),
                                ALU.add,
                            )

        if debug_dumps:
            nc.sync.dma_start(dbg_qT, qT)
            nc.sync.dma_start(dbg_kT, kT)
            nc.sync.dma_start(dbg_vaug, v_aug)

        # ---- attention + output projection ----
        with tc.tile_pool(name="sc_ps", bufs=4, space="PSUM") as sc_ps, \
             tc.tile_pool(name="pv_ps", bufs=2, space="PSUM") as pv_pool, \
             tc.tile_pool(name="rb_ps", bufs=1, space="PSUM") as rb_pool, \
             tc.tile_pool(name="op_ps", bufs=1, space="PSUM") as op_ps, \
             tc.tile_pool(name="probs", bufs=6) as probs_pool, \
             tc.tile_pool(name="attn", bufs=M_CH + 1) as attn_pool, \
             tc.tile_pool(name="misc", bufs=3) as misc, \
             tc.tile_pool(name="aupool", bufs=2 * M_CH + 1) as aupool, \
             tc.tile_pool(name="outst", bufs=3) as outst:
            for qb in range(N_QB):
                attn_tiles = []
                mq_work = []
                sums_g = misc.tile([2 * M_CH, QB], F32, tag="sums_g")
                recips_g = misc.tile([2 * M_CH, QB], F32R, tag="recips_g")
                for m in range(M_CH):
                    n_kc = (qb + 1) * (QB // KC) if causal else N_KC
                    pv_t = [pv_pool.tile([65, QB], F32, tag="pv", name=f"pv{hl}")
                             for hl in (0, 1)]
                    for kc in range(n_kc):
                        r = kc - (n_kc - R)
                        is_diag = causal and r >= 0
                        ps_h = [sc_ps.tile([P, QB], F32, tag="sc",
                                          name=f"sc{hl}")
                                for hl in (0, 1)]
                        for hl in (0, 1):
                            rows = slice(64 * hl, 64 * hl + 64)
                            nc.tensor.matmul(
                                ps_h[hl],
                                lhsT=kT[rows, m, kc * KC:(kc + 1) * KC],
                                rhs=qT[rows, m, qb * QB:(qb + 1) * QB],
                                start=True, stop=True,
                            )
                        for hl in (0, 1):
                            pt = probs_pool.tile([P, QB], DT, tag="pt")
                            nc.scalar.activation(pt, ps_h[hl], AF.Exp,
                                                 scale=float(SCALE))
                            if is_diag:
                                nc.vector.tensor_tensor(
                                    pt, pt, dmask[:, r, :], ALU.mult)
                            if (debug_dumps and qb == 0 and m == 0
                                    and kc == 0 and hl == 0):
                                nc.sync.dma_start(dbg_probs, pt)
                            nc.tensor.matmul(
                                pv_t[hl],
                                lhsT=v_aug[:, kc, 2 * m + hl, :],
                                rhs=pt,
                                start=(kc == 0), stop=(kc == n_kc - 1),
                            )
                    # Drain PSUM fast: copy attn + sums out so PV tiles
                    # recycle; stash sums rows for the per-qb batched recip.
                    attn_us = []
                    for hl in (0, 1):
                        attn_u = aupool.tile([64, QB], DT, tag="attn_u",
                                             name=f"attn_u{hl}")
                        nc.any.tensor_copy(attn_u, pv_t[hl][0:64, :])
                        attn_us.append(attn_u)
                        sums_sb = misc.tile([65, QB], F32, tag="sums_sb",
                                            name=f"sums{hl}")
                        nc.any.tensor_copy(sums_sb[64:65, :],
                                           pv_t[hl][64:65, :])
                        nc.sync.dma_start(sums_g[2 * m + hl: 2 * m + hl + 1],
                                          sums_sb[64:65, :])
                    mq_work.append((m, attn_us))
                for m, attn_us in mq_work:
                    if m == 0:
                        # one 8-lane reciprocal for all (m, hl) of this qb
                        with nc.allow_low_precision(
                                reason="softmax denom recip, f32r rounding"):
                            nc.vector.reciprocal(recips_g, sums_g)
                    attn_m = attn_pool.tile([P, QB], DT, tag="attn")
                    for hl in (0, 1):
                        recip65 = misc.tile([65, QB], F32R, tag="recip")
                        nc.sync.dma_start(recip65[64:65, :],
                                          recips_g[2 * m + hl: 2 * m + hl + 1])
                        rb = rb_pool.tile([64, QB], F32, tag="rb")
                        nc.tensor.matmul(rb, lhsT=ones65[64:65, :],
                                         rhs=recip65[64:65, :],
                                         start=True, stop=True)
                        nc.vector.tensor_tensor(
                            attn_m[64 * hl:64 * hl + 64, :], attn_us[hl], rb,
                            ALU.mult)
                    if debug_dumps and qb == 0:
                        nc.sync.dma_start(dbg_attn[m], attn_m)
                    attn_tiles.append(attn_m)
                for ssub in range(QB // P):
                    for nout in range(O_N):
                        pso = op_ps.tile([P, 512], F32, tag="op")
                        for m in range(M_CH):
                            nc.tensor.matmul(
                                pso,
                                lhsT=attn_tiles[m][:, ssub * P:(ssub + 1) * P],
                                rhs=wo_sb[:, m, nout * 512:(nout + 1) * 512],
                                start=(m == 0), stop=(m == M_CH - 1),
                            )
                        st = outst.tile([P, 512], F32, tag="st")
                        nc.any.tensor_copy(st, pso)
                        nc.sync.dma_start(
                            out_p[qb * QB + ssub * P: qb * QB + (ssub + 1) * P,
                                  nout * 512:(nout + 1) * 512],
                            st)
    nc.compile()
    return nc


def make_consts(S, use_bf16):
    """Host-built 0/1 causal masks for the R diagonal key-chunks."""
    d = _dims(S)
    QB, R = d["QB"], d["R"]
    npdt = _np_dt(use_bf16)
    i = np.arange(P)[:, None]
    j = np.arange(QB)[None, :]
    dmask = np.stack([(i <= j - KC * r) for r in range(R)], 1)
    return dmask.astype(npdt)


def core_inputs(Q, K, V, W_q, b_q, W_k, b_k, W_v, b_v, W_o, b, hg, S, use_bf16):
    """Build the per-core input map (host-side slicing/transposition/casts)."""
    npdt = _np_dt(use_bf16)
    d = _dims(S)
    M_CH = d["M_CH"]
    rows = slice(hg * DH, (hg + 1) * DH)

    def t(x):
        return np.ascontiguousarray(np.asarray(x, np.float32).T).astype(npdt)

    dmask = make_consts(S, use_bf16)
    return {
        "xq_t": t(Q[b]), "xk_t": t(K[b]), "xv_t": t(V[b]),
        "wq_t": t(W_q[rows]), "wk_t": t(W_k[rows]), "wv_t": t(W_v[rows]),
        "wo_t": t(W_o[:, rows]),
        "bq_p": np.ascontiguousarray(
            np.asarray(b_q[rows], np.float32).reshape(M_CH, P).T),
        "bk_p": np.ascontiguousarray(
            np.asarray(b_k[rows], np.float32).reshape(M_CH, P).T),
        "bv_r": np.broadcast_to(
            np.asarray(b_v[rows], np.float32), (P, DH)).copy(),
        "dmask": dmask,
        "ones_c": np.ones((65, 64), np.float32),
        "ones_v": np.ones((P, d["N_KC"], NH_G, 1), npdt),
    }


def _np_reference(Q, K, V, mask, W_q, b_q, W_k, b_k, W_v, b_v, W_o, b_o):
    """Exact numpy fallback for arbitrary masks."""
    q = (Q @ W_q.T + b_q).reshape(B, S_FULL, H, DK).transpose(0, 2, 1, 3)
    k = (K @ W_k.T + b_k).reshape(B, S_FULL, H, DK).transpose(0, 2, 1, 3)
    v = (V @ W_v.T + b_v).reshape(B, S_FULL, H, DK).transpose(0, 2, 1, 3)
    scores = np.einsum("bhqd,bhkd->bhqk", q, k) / np.sqrt(np.float32(DK))
    scores = np.where(mask == 0, np.finfo(np.float32).min, scores)
    scores -= scores.max(-1, keepdims=True)
    probs = np.exp(scores)
    probs /= probs.sum(-1, keepdims=True)
    out = np.einsum("bhqk,bhkd->bhqd", probs, v)
    out = out.transpose(0, 2, 1, 3).reshape(B, S_FULL, D)
    return (out @ W_o.T + b_o).astype(np.float32)


def kernel(Q, K, V, mask, W_q, b_q, W_k, b_k, W_v, b_v, W_o, b_o):
    Q = np.asarray(Q, np.float32)
    K = np.asarray(K, np.float32)
    V = np.asarray(V, np.float32)
    mask = np.asarray(mask)

    m2 = mask.reshape(mask.shape[-2], mask.shape[-1])
    if np.array_equal(m2 != 0, np.tril(np.ones(m2.shape, bool))):
        causal = True
    elif (m2 != 0).all():
        causal = False
    else:
        return _np_reference(Q, K, V, mask, W_q, b_q, W_k, b_k, W_v, b_v,
                             W_o, b_o)

    use_bf16 = os.environ.get("MHA_KERNEL_DTYPE", "f32r") == "bf16"
    import concourse.bass_utils as _bu
    from concourse.bass_utils import run_bass_kernel_spmd
    if (os.environ.get("MHA_LDW_OPT", "0") == "1"
            and not getattr(_bu, "_mha_ldw_patched", False)):
        _orig_rc = _bu.run_command

        def _rc(argv, **kw):
            argv = ["--enable-ldw-opt=true" if a == "--enable-ldw-opt=false"
                    else a for a in argv]
            return _orig_rc(argv, **kw)

        _bu.run_command = _rc
        _bu._mha_ldw_patched = True

    key = (causal, S_FULL, use_bf16)
    if key not in _PROG_CACHE:
        _PROG_CACHE[key] = build_program(causal, S_FULL, use_bf16)
    nc = _PROG_CACHE[key]

    in_maps = []
    for c in range(8):
        b, hg = divmod(c, 2)
        in_maps.append(core_inputs(Q, K, V, W_q, b_q, W_k, b_k, W_v, b_v,
                                   W_o, b, hg, S_FULL, use_bf16))

    trace = os.environ.get("MHA_KERNEL_TRACE", "0") == "1"
    kw = {}
    if trace:
        kw = {"trace": True,
              "trace_cores": [int(x) for x in os.environ.get(
                  "MHA_TRACE_CORES", "0").split(",")]}
    n_cores = int(os.environ.get("MHA_CORES", "8"))
    res = run_bass_kernel_spmd(nc, in_maps[:n_cores],
                               core_ids=list(range(n_cores)), **kw)
    kernel.last_results = res

    b_o32 = np.asarray(b_o, np.float32)
    out = np.zeros((B, S_FULL, D), np.float32)
    for b in range(B):
        if 2 * b + 1 < n_cores:
            out[b] = (res.results[2 * b]["out_p"]
                      + res.results[2 * b + 1]["out_p"] + b_o32[None, :])
    return out


kernel.last_results = None


# revision 21
# speedup vs baseline: 1.7146x; 1.0204x over previous
"""MultiHeadAttention Trainium2 kernel (8 NeuronCores).

Sharding: core c handles batch b = c // 2 and head-group hg = c % 2
(8 of 16 heads, 512 of 1024 model dims). Attention is embarrassingly
parallel over (b, hg); the output projection is computed per head-group
against the matching W_o columns, yielding partial outputs that the host
sums (plus b_o).

Device dataflow (per core), all in "transposed" layouts so no on-device
transposes are ever needed:
  qT = Wq_hg @ Xq^T      [dh=512, S]   (lhsT = Wq_hg^T, rhs = Xq^T; host preps both)
  kT = Wk_hg @ Xk^T      [dh=512, S]
  v  = Xv @ Wv_hg^T      [S, dh=512]   (+ ones column per head for softmax sums)
  scores_T[k, q] = kT_h[:, kchunk]^T-matmul  (keys on partitions)
  causal mask: extra PE matmul tri^T @ step accumulating -1e9 into masked entries
  probs = exp(scores_T / 8) on ACT (no max subtraction: scores ~ N(0,1), safe)
  attn_T[d, q] (+ sums row) = v_chunk^T-matmul over probs, accumulated in PSUM
  normalize: recip = 1/sums (DVE), broadcast via ones-matmul, multiply (DVE)
  out_partial = attn^T-matmul with Wo columns
"""

import os

import numpy as np

B, S_FULL, D = 4, 2048, 1024
H, DK = 16, 64
NH_G = 8          # heads per core
DH = NH_G * DK    # 512 dims per core
P = 128
KC = 128          # key chunk (PE contraction)
NEG = -1.0e9
SCALE = 1.0 / np.sqrt(np.float32(DK))

_PROG_CACHE = {}


def _dims(S):
    QB = min(512, S)
    return {
        "S": S, "QB": QB, "N_QB": S // QB, "N_KC": S // KC,
        "R": QB // KC, "E_CH": D // P, "M_CH": DH // P, "O_N": D // 512,
    }


def _np_dt(use_bf16):
    if use_bf16:
        import ml_dtypes
        return ml_dtypes.bfloat16
    return np.float32


def build_program(causal, S, use_bf16, debug_dumps=False):
    """Build the single-core Bass/Tile program (same program on all 8 cores)."""
    from contextlib import ExitStack

    import concourse.bass as bass
    import concourse.tile as tile
    from concourse import bacc, mybir

    d = _dims(S)
    QB, N_QB, N_KC, R, E_CH, M_CH, O_N = (
        d["QB"], d["N_QB"], d["N_KC"], d["R"], d["E_CH"], d["M_CH"], d["O_N"])

    DT = mybir.dt.bfloat16 if use_bf16 else mybir.dt.float32r
    F32 = mybir.dt.float32
    F32R = mybir.dt.float32r
    AF = mybir.ActivationFunctionType
    ALU = mybir.AluOpType

    nc = bacc.Bacc("TRN2", target_bir_lowering=False, debug=False)

    NB = S // QB
    xq_t = nc.dram_tensor("xq_t", [NB, P, E_CH, QB], DT,
                          kind="ExternalInput").ap()
    xk_t = nc.dram_tensor("xk_t", [NB, P, E_CH, QB], DT,
                          kind="ExternalInput").ap()
    xv_t = nc.dram_tensor("xv_t", [NB, P, E_CH, QB], DT,
                          kind="ExternalInput").ap()
    wq_t = nc.dram_tensor("wq_t", [P, E_CH, DH], DT,
                          kind="ExternalInput").ap()
    wk_t = nc.dram_tensor("wk_t", [P, E_CH, DH], DT,
                          kind="ExternalInput").ap()
    wv_t = nc.dram_tensor("wv_t", [P, E_CH, DH], DT,
                          kind="ExternalInput").ap()
    wo_t = nc.dram_tensor("wo_t", [P, M_CH, D], DT,
                          kind="ExternalInput").ap()
    bq_in = nc.dram_tensor("bq_p", [P, M_CH], F32, kind="ExternalInput").ap()
    bk_in = nc.dram_tensor("bk_p", [P, M_CH], F32, kind="ExternalInput").ap()
    bv_in = nc.dram_tensor("bv_r", [P, DH], F32, kind="ExternalInput").ap()
    dmask_in = nc.dram_tensor("dmask", [P, R, QB], DT,
                              kind="ExternalInput").ap()
    ones_c_in = nc.dram_tensor("ones_c", [65, 64], F32R,
                               kind="ExternalInput").ap()
    ones_v_in = nc.dram_tensor("ones_v", [P, N_KC, NH_G, 1], DT,
                               kind="ExternalInput").ap()
    out_p = nc.dram_tensor("out_p", [S, D], F32, kind="ExternalOutput").ap()
    if debug_dumps:
        dbg_qT = nc.dram_tensor("dbg_qT", [P, M_CH, S], DT,
                                kind="ExternalOutput").ap()
        dbg_kT = nc.dram_tensor("dbg_kT", [P, M_CH, S], DT,
                                kind="ExternalOutput").ap()
        dbg_vaug = nc.dram_tensor("dbg_vaug", [P, N_KC, NH_G, 65], DT,
                                  kind="ExternalOutput").ap()
        dbg_probs = nc.dram_tensor("dbg_probs", [P, QB], DT,
                                   kind="ExternalOutput").ap()
        dbg_attn = nc.dram_tensor("dbg_attn", [M_CH, P, QB], DT,
                                  kind="ExternalOutput").ap()
        dbg_recip = nc.dram_tensor("dbg_recip", [P, 3, QB], F32,
                                   kind="ExternalOutput").ap()

    with tile.TileContext(nc) as tc, ExitStack() as ctx:
        consts = ctx.enter_context(tc.tile_pool(name="consts", bufs=1))
        wpool = ctx.enter_context(tc.tile_pool(name="w", bufs=2))
        qkv = ctx.enter_context(tc.tile_pool(name="qkv", bufs=1))

        dmask = consts.tile([P, R, QB], DT)
        nc.sync.dma_start(dmask, dmask_in)
        bq_sb = consts.tile([P, M_CH], F32)
        nc.sync.dma_start(bq_sb, bq_in)
        bk_sb = consts.tile([P, M_CH], F32)
        nc.sync.dma_start(bk_sb, bk_in)
        bv_sb = consts.tile([P, DH], F32)
        nc.sync.dma_start(bv_sb, bv_in)
        ones65 = consts.tile([65, 64], F32R)
        nc.sync.dma_start(ones65, ones_c_in)

        qT = qkv.tile([P, M_CH, S], DT, tag="qT")
        kT = qkv.tile([P, M_CH, S], DT, tag="kT")
        v_aug = qkv.tile([P, N_KC, NH_G, 65], DT, tag="v_aug")
        nc.sync.dma_start(v_aug[:, :, :, 64:65], ones_v_in)

        w_tiles = {}
        for name, src in (("wq", wq_t), ("wk", wk_t), ("wv", wv_t)):
            w_sb = wpool.tile([P, E_CH, DH], DT, tag="w")
            for e in range(E_CH):
                nc.sync.dma_start(w_sb[:, e], src[:, e])
            w_tiles[name] = w_sb
        wo_sb = wpool.tile([P, M_CH, D], DT, tag="w")
        nc.sync.dma_start(wo_sb, wo_t)

        # ---- projections ----
        with tc.tile_pool(name="xp", bufs=3) as xpool, \
             tc.tile_pool(name="pj", bufs=3, space="PSUM") as pj_ps:
            for phase, x_in, w_sb, b_sb in (
                ("q", xq_t, w_tiles["wq"], bq_sb),
                ("k", xk_t, w_tiles["wk"], bk_sb),
                ("v", xv_t, w_tiles["wv"], bv_sb),
            ):
                dst = qT if phase == "q" else kT
                for n in range(N_QB):
                    xblk = xpool.tile([P, E_CH, QB], DT, tag="x")
                    for e in range(E_CH):
                        nc.sync.dma_start(xblk[:, e], x_in[n, :, e])
                    if phase in ("q", "k"):
                        for m in range(M_CH):
                            ps = pj_ps.tile([P, QB], F32, tag="pj")
                            for e in range(E_CH):
                                nc.tensor.matmul(
                                    ps,
                                    lhsT=w_sb[:, e, m * P:(m + 1) * P],
                                    rhs=xblk[:, e, :],
                                    start=(e == 0), stop=(e == E_CH - 1),
                                )
                            nc.vector.tensor_scalar_add(
                                dst[:, m, n * QB:(n + 1) * QB], ps,
                                b_sb[:, m:m + 1])
                    else:
                        for sc in range(QB // P):
                            ps = pj_ps.tile([P, DH], F32, tag="pj")
                            for e in range(E_CH):
                                nc.tensor.matmul(
                                    ps,
                                    lhsT=xblk[:, e, sc * P:(sc + 1) * P],
                                    rhs=w_sb[:, e, :],
                                    start=(e == 0), stop=(e == E_CH - 1),
                                )
                            kc = n * (QB // P) + sc
                            nc.vector.tensor_tensor(
                                v_aug[:, kc, :, 0:64],
                                ps.rearrange("p (h e) -> p h e", h=NH_G),
                                bv_sb.rearrange("p (h e) -> p h e", h=NH_G),
                                ALU.add,
                            )

        if debug_dumps:
            nc.sync.dma_start(dbg_qT, qT)
            nc.sync.dma_start(dbg_kT, kT)
            nc.sync.dma_start(dbg_vaug, v_aug)

        # ---- attention + output projection ----
        with tc.tile_pool(name="sc_ps", bufs=4, space="PSUM") as sc_ps, \
             tc.tile_pool(name="pv_ps", bufs=2, space="PSUM") as pv_pool, \
             tc.tile_pool(name="rb_ps", bufs=1, space="PSUM") as rb_pool, \
             tc.tile_pool(name="op_ps", bufs=1, space="PSUM") as op_ps, \
             tc.tile_pool(name="probs", bufs=6) as probs_pool, \
             tc.tile_pool(name="attn", bufs=M_CH + 1) as attn_pool, \
             tc.tile_pool(name="misc", bufs=3) as misc, \
             tc.tile_pool(name="aupool", bufs=2 * M_CH + 1) as aupool, \
             tc.tile_pool(name="outst", bufs=3) as outst:
            for qb in range(N_QB):
                attn_tiles = []
                mq_work = []
                sums_g = misc.tile([2 * M_CH, QB], F32, tag="sums_g")
                recips_g = misc.tile([2 * M_CH, QB], F32R, tag="recips_g")
                for m in range(M_CH):
                    n_kc = (qb + 1) * (QB // KC) if causal else N_KC
                    pv_t = [pv_pool.tile([65, QB], F32, tag="pv", name=f"pv{hl}")
                             for hl in (0, 1)]
                    for kc in range(n_kc):
                        r = kc - (n_kc - R)
                        is_diag = causal and r >= 0
                        ps_h = [sc_ps.tile([P, QB], F32, tag="sc",
                                          name=f"sc{hl}")
                                for hl in (0, 1)]
                        for hl in (0, 1):
                            rows = slice(64 * hl, 64 * hl + 64)
                            nc.tensor.matmul(
                                ps_h[hl],
                                lhsT=kT[rows, m, kc * KC:(kc + 1) * KC],
                                rhs=qT[rows, m, qb * QB:(qb + 1) * QB],
                                start=True, stop=True,
                            )
                        for hl in (0, 1):
                            pt = probs_pool.tile([P, QB], DT, tag="pt")
                            c0 = KC * r if is_diag and r > 0 else 0
                            if c0:
                                nc.gpsimd.memset(pt[:, 0:c0], 0.0)
                            nc.scalar.activation(pt[:, c0:], ps_h[hl][:, c0:],
                                                 AF.Exp, scale=float(SCALE))
                            if is_diag:
                                nc.vector.tensor_tensor(
                                    pt[:, c0:], pt[:, c0:],
                                    dmask[:, r, c0:], ALU.mult)
                            if (debug_dumps and qb == 0 and m == 0
                                    and kc == 0 and hl == 0):
                                nc.sync.dma_start(dbg_probs, pt)
                            nc.tensor.matmul(
                                pv_t[hl],
                                lhsT=v_aug[:, kc, 2 * m + hl, :],
                                rhs=pt,
                                start=(kc == 0), stop=(kc == n_kc - 1),
                            )
                    # Drain PSUM fast: copy attn + sums out so PV tiles
                    # recycle; stash sums rows for the per-qb batched recip.
                    attn_us = []
                    for hl in (0, 1):
                        attn_u = aupool.tile([64, QB], DT, tag="attn_u",
                                             name=f"attn_u{hl}")
                        nc.any.tensor_copy(attn_u, pv_t[hl][0:64, :])
                        attn_us.append(attn_u)
                        sums_sb = misc.tile([65, QB], F32, tag="sums_sb",
                                            name=f"sums{hl}")
                        nc.any.tensor_copy(sums_sb[64:65, :],
                                           pv_t[hl][64:65, :])
                        nc.sync.dma_start(sums_g[2 * m + hl: 2 * m + hl + 1],
                                          sums_sb[64:65, :])
                    mq_work.append((m, attn_us))
                for m, attn_us in mq_work:
                    if m == 0:
                        # one 8-lane reciprocal for all (m, hl) of this qb
                        with nc.allow_low_precision(
                                reason="softmax denom recip, f32r rounding"):
                            nc.vector.reciprocal(recips_g, sums_g)
                    attn_m = attn_pool.tile([P, QB], DT, tag="attn")
                    for hl in (0, 1):
                        recip65 = misc.tile([65, QB], F32R, tag="recip")
                        nc.sync.dma_start(recip65[64:65, :],
                                          recips_g[2 * m + hl: 2 * m + hl + 1])
                        rb = rb_pool.tile([64, QB], F32, tag="rb")
                        nc.tensor.matmul(rb, lhsT=ones65[64:65, :],
                                         rhs=recip65[64:65, :],
                                         start=True, stop=True)
                        nc.vector.tensor_tensor(
                            attn_m[64 * hl:64 * hl + 64, :], attn_us[hl], rb,
                            ALU.mult)
                    if debug_dumps and qb == 0:
                        nc.sync.dma_start(dbg_attn[m], attn_m)
                    attn_tiles.append(attn_m)
                for ssub in range(QB // P):
                    for nout in range(O_N):
                        pso = op_ps.tile([P, 512], F32, tag="op")
                        for m in range(M_CH):
                            nc.tensor.matmul(
                                pso,
                                lhsT=attn_tiles[m][:, ssub * P:(ssub + 1) * P],
                                rhs=wo_sb[:, m, nout * 512:(nout + 1) * 512],
                                start=(m == 0), stop=(m == M_CH - 1),
                            )
                        st = outst.tile([P, 512], F32, tag="st")
                        nc.any.tensor_copy(st, pso)
                        nc.sync.dma_start(
                            out_p[qb * QB + ssub * P: qb * QB + (ssub + 1) * P,
                                  nout * 512:(nout + 1) * 512],
                            st)
    nc.compile()
    return nc


def make_consts(S, use_bf16):
    """Host-built 0/1 causal masks for the R diagonal key-chunks."""
    d = _dims(S)
    QB, R = d["QB"], d["R"]
    npdt = _np_dt(use_bf16)
    i = np.arange(P)[:, None]
    j = np.arange(QB)[None, :]
    dmask = np.stack([(i <= j - KC * r) for r in range(R)], 1)
    return dmask.astype(npdt)


def core_inputs(Q, K, V, W_q, b_q, W_k, b_k, W_v, b_v, W_o, b, hg, S, use_bf16):
    """Build the per-core input map (host-side slicing/transposition/casts)."""
    npdt = _np_dt(use_bf16)
    d = _dims(S)
    M_CH = d["M_CH"]
    rows = slice(hg * DH, (hg + 1) * DH)

    QB = d["QB"]
    E_CH = D // P

    def xt(x):
        # [S, D] -> [N_QB, P, E_CH, QB]: per-core x, transposed and tiled so
        # each device block load is a contiguous DMA.
        a = np.asarray(x, np.float32).T.astype(npdt)      # [D, S]
        a = a.reshape(E_CH, P, S // QB, QB).transpose(2, 1, 0, 3)
        return np.ascontiguousarray(a)

    def wt(w):
        # [DH, D] slice -> W^T tiled [P, E_CH, DH]
        a = np.asarray(w, np.float32).T.astype(npdt)      # [D, DH]
        return np.ascontiguousarray(
            a.reshape(E_CH, P, DH).transpose(1, 0, 2))

    def wot(w):
        # W_o[:, rows] -> [P, M_CH, D]
        a = np.asarray(w, np.float32).T.astype(npdt)      # [DH, D]... wait
        return a

    a_wo = np.asarray(W_o[:, rows], np.float32).T.astype(npdt)  # [DH, D]
    wo_prep = np.ascontiguousarray(
        a_wo.reshape(M_CH, P, D).transpose(1, 0, 2))

    dmask = make_consts(S, use_bf16)
    return {
        "xq_t": xt(Q[b]), "xk_t": xt(K[b]), "xv_t": xt(V[b]),
        "wq_t": wt(W_q[rows]), "wk_t": wt(W_k[rows]), "wv_t": wt(W_v[rows]),
        "wo_t": wo_prep,
        "bq_p": np.ascontiguousarray(
            np.asarray(b_q[rows], np.float32).reshape(M_CH, P).T),
        "bk_p": np.ascontiguousarray(
            np.asarray(b_k[rows], np.float32).reshape(M_CH, P).T),
        "bv_r": np.broadcast_to(
            np.asarray(b_v[rows], np.float32), (P, DH)).copy(),
        "dmask": dmask,
        "ones_c": np.ones((65, 64), np.float32),
        "ones_v": np.ones((P, d["N_KC"], NH_G, 1), npdt),
    }


def _np_reference(Q, K, V, mask, W_q, b_q, W_k, b_k, W_v, b_v, W_o, b_o):
    """Exact numpy fallback for arbitrary masks."""
    q = (Q @ W_q.T + b_q).reshape(B, S_FULL, H, DK).transpose(0, 2, 1, 3)
    k = (K @ W_k.T + b_k).reshape(B, S_FULL, H, DK).transpose(0, 2, 1, 3)
    v = (V @ W_v.T + b_v).reshape(B, S_FULL, H, DK).transpose(0, 2, 1, 3)
    scores = np.einsum("bhqd,bhkd->bhqk", q, k) / np.sqrt(np.float32(DK))
    scores = np.where(mask == 0, np.finfo(np.float32).min, scores)
    scores -= scores.max(-1, keepdims=True)
    probs = np.exp(scores)
    probs /= probs.sum(-1, keepdims=True)
    out = np.einsum("bhqk,bhkd->bhqd", probs, v)
    out = out.transpose(0, 2, 1, 3).reshape(B, S_FULL, D)
    return (out @ W_o.T + b_o).astype(np.float32)


def kernel(Q, K, V, mask, W_q, b_q, W_k, b_k, W_v, b_v, W_o, b_o):
    Q = np.asarray(Q, np.float32)
    K = np.asarray(K, np.float32)
    V = np.asarray(V, np.float32)
    mask = np.asarray(mask)

    m2 = mask.reshape(mask.shape[-2], mask.shape[-1])
    if np.array_equal(m2 != 0, np.tril(np.ones(m2.shape, bool))):
        causal = True
    elif (m2 != 0).all():
        causal = False
    else:
        return _np_reference(Q, K, V, mask, W_q, b_q, W_k, b_k, W_v, b_v,
                             W_o, b_o)

    use_bf16 = os.environ.get("MHA_KERNEL_DTYPE", "f32r") == "bf16"
    import concourse.bass_utils as _bu
    from concourse.bass_utils import run_bass_kernel_spmd
    if (os.environ.get("MHA_LDW_OPT", "0") == "1"
            and not getattr(_bu, "_mha_ldw_patched", False)):
        _orig_rc = _bu.run_command

        def _rc(argv, **kw):
            argv = ["--enable-ldw-opt=true" if a == "--enable-ldw-opt=false"
                    else a for a in argv]
            return _orig_rc(argv, **kw)

        _bu.run_command = _rc
        _bu._mha_ldw_patched = True

    key = (causal, S_FULL, use_bf16)
    if key not in _PROG_CACHE:
        _PROG_CACHE[key] = build_program(causal, S_FULL, use_bf16)
    nc = _PROG_CACHE[key]

    in_maps = []
    for c in range(8):
        b, hg = divmod(c, 2)
        in_maps.append(core_inputs(Q, K, V, W_q, b_q, W_k, b_k, W_v, b_v,
                                   W_o, b, hg, S_FULL, use_bf16))

    trace = os.environ.get("MHA_KERNEL_TRACE", "0") == "1"
    kw = {}
    if trace:
        kw = {"trace": True,
              "trace_cores": [int(x) for x in os.environ.get(
                  "MHA_TRACE_CORES", "0").split(",")]}
    n_cores = int(os.environ.get("MHA_CORES", "8"))
    res = run_bass_kernel_spmd(nc, in_maps[:n_cores],
                               core_ids=list(range(n_cores)), **kw)
    kernel.last_results = res

    b_o32 = np.asarray(b_o, np.float32)
    out = np.zeros((B, S_FULL, D), np.float32)
    for b in range(B):
        if 2 * b + 1 < n_cores:
            out[b] = (res.results[2 * b]["out_p"]
                      + res.results[2 * b + 1]["out_p"] + b_o32[None, :])
    return out


kernel.last_results = None


# revision 22
# speedup vs baseline: 1.7430x; 1.0165x over previous
"""MultiHeadAttention Trainium2 kernel (8 NeuronCores).

Sharding: core c handles batch b = c // 2 and head-group hg = c % 2
(8 of 16 heads, 512 of 1024 model dims). Attention is embarrassingly
parallel over (b, hg); the output projection is computed per head-group
against the matching W_o columns, yielding partial outputs that the host
sums (plus b_o).

Device dataflow (per core), all in "transposed" layouts so no on-device
transposes are ever needed:
  qT = Wq_hg @ Xq^T      [dh=512, S]   (lhsT = Wq_hg^T, rhs = Xq^T; host preps both)
  kT = Wk_hg @ Xk^T      [dh=512, S]
  v  = Xv @ Wv_hg^T      [S, dh=512]   (+ ones column per head for softmax sums)
  scores_T[k, q] = kT_h[:, kchunk]^T-matmul  (keys on partitions)
  causal mask: extra PE matmul tri^T @ step accumulating -1e9 into masked entries
  probs = exp(scores_T / 8) on ACT (no max subtraction: scores ~ N(0,1), safe)
  attn_T[d, q] (+ sums row) = v_chunk^T-matmul over probs, accumulated in PSUM
  normalize: recip = 1/sums (DVE), broadcast via ones-matmul, multiply (DVE)
  out_partial = attn^T-matmul with Wo columns
"""

import os

import numpy as np

B, S_FULL, D = 4, 2048, 1024
H, DK = 16, 64
NH_G = 8          # heads per core
DH = NH_G * DK    # 512 dims per core
P = 128
KC = 128          # key chunk (PE contraction)
NEG = -1.0e9
SCALE = 1.0 / np.sqrt(np.float32(DK))

_PROG_CACHE = {}


def _dims(S):
    QB = min(512, S)
    return {
        "S": S, "QB": QB, "N_QB": S // QB, "N_KC": S // KC,
        "R": QB // KC, "E_CH": D // P, "M_CH": DH // P, "O_N": D // 512,
    }


def _np_dt(use_bf16):
    if use_bf16:
        import ml_dtypes
        return ml_dtypes.bfloat16
    return np.float32


def build_program(causal, S, use_bf16, debug_dumps=False):
    """Build the single-core Bass/Tile program (same program on all 8 cores)."""
    from contextlib import ExitStack

    import concourse.bass as bass
    import concourse.tile as tile
    from concourse import bacc, mybir

    d = _dims(S)
    QB, N_QB, N_KC, R, E_CH, M_CH, O_N = (
        d["QB"], d["N_QB"], d["N_KC"], d["R"], d["E_CH"], d["M_CH"], d["O_N"])

    DT = mybir.dt.bfloat16 if use_bf16 else mybir.dt.float32r
    F32 = mybir.dt.float32
    F32R = mybir.dt.float32r
    AF = mybir.ActivationFunctionType
    ALU = mybir.AluOpType

    nc = bacc.Bacc("TRN2", target_bir_lowering=False, debug=False)

    NB = S // QB
    xq_t = nc.dram_tensor("xq_t", [NB, P, E_CH, QB], DT,
                          kind="ExternalInput").ap()
    xk_t = nc.dram_tensor("xk_t", [NB, P, E_CH, QB], DT,
                          kind="ExternalInput").ap()
    xv_t = nc.dram_tensor("xv_t", [NB, P, E_CH, QB], DT,
                          kind="ExternalInput").ap()
    wq_t = nc.dram_tensor("wq_t", [P, E_CH, DH], DT,
                          kind="ExternalInput").ap()
    wk_t = nc.dram_tensor("wk_t", [P, E_CH, DH], DT,
                          kind="ExternalInput").ap()
    wv_t = nc.dram_tensor("wv_t", [P, E_CH, DH], DT,
                          kind="ExternalInput").ap()
    wo_t = nc.dram_tensor("wo_t", [P, M_CH, D], DT,
                          kind="ExternalInput").ap()
    bq_in = nc.dram_tensor("bq_p", [P, M_CH], F32, kind="ExternalInput").ap()
    bk_in = nc.dram_tensor("bk_p", [P, M_CH], F32, kind="ExternalInput").ap()
    bv_in = nc.dram_tensor("bv_r", [P, DH], F32, kind="ExternalInput").ap()
    dmask_in = nc.dram_tensor("dmask", [P, R, QB], DT,
                              kind="ExternalInput").ap()
    ones_c_in = nc.dram_tensor("ones_c", [65, 64], F32R,
                               kind="ExternalInput").ap()
    ones_v_in = nc.dram_tensor("ones_v", [P, N_KC, NH_G, 1], DT,
                               kind="ExternalInput").ap()
    out_p = nc.dram_tensor("out_p", [S, D], F32, kind="ExternalOutput").ap()
    if debug_dumps:
        dbg_qT = nc.dram_tensor("dbg_qT", [P, M_CH, S], DT,
                                kind="ExternalOutput").ap()
        dbg_kT = nc.dram_tensor("dbg_kT", [P, M_CH, S], DT,
                                kind="ExternalOutput").ap()
        dbg_vaug = nc.dram_tensor("dbg_vaug", [P, N_KC, NH_G, 65], DT,
                                  kind="ExternalOutput").ap()
        dbg_probs = nc.dram_tensor("dbg_probs", [P, QB], DT,
                                   kind="ExternalOutput").ap()
        dbg_attn = nc.dram_tensor("dbg_attn", [M_CH, P, QB], DT,
                                  kind="ExternalOutput").ap()
        dbg_recip = nc.dram_tensor("dbg_recip", [P, 3, QB], F32,
                                   kind="ExternalOutput").ap()

    with tile.TileContext(nc) as tc, ExitStack() as ctx:
        consts = ctx.enter_context(tc.tile_pool(name="consts", bufs=1))
        wpool = ctx.enter_context(tc.tile_pool(name="w", bufs=2))
        qkv = ctx.enter_context(tc.tile_pool(name="qkv", bufs=1))

        dmask = consts.tile([P, R, QB], DT)
        nc.sync.dma_start(dmask, dmask_in)
        bq_sb = consts.tile([P, M_CH], F32)
        nc.sync.dma_start(bq_sb, bq_in)
        bk_sb = consts.tile([P, M_CH], F32)
        nc.sync.dma_start(bk_sb, bk_in)
        bv_sb = consts.tile([P, DH], F32)
        nc.sync.dma_start(bv_sb, bv_in)
        ones65 = consts.tile([65, 64], F32R)
        nc.sync.dma_start(ones65, ones_c_in)

        qT = qkv.tile([P, M_CH, S], DT, tag="qT")
        kT = qkv.tile([P, M_CH, S], DT, tag="kT")
        v_aug = qkv.tile([P, N_KC, NH_G, 65], DT, tag="v_aug")
        if use_bf16:
            # strided single-element DMA is ~10us and blocks the queue;
            # gpsimd memset handles bf16 fine
            nc.gpsimd.memset(v_aug[:, :, :, 64:65], 1.0)
        else:
            nc.gpsimd.dma_start(v_aug[:, :, :, 64:65], ones_v_in)

        w_tiles = {}
        for name, src in (("wq", wq_t), ("wk", wk_t), ("wv", wv_t)):
            w_sb = wpool.tile([P, E_CH, DH], DT, tag="w")
            for e in range(E_CH):
                nc.sync.dma_start(w_sb[:, e], src[:, e])
            w_tiles[name] = w_sb
        wo_sb = wpool.tile([P, M_CH, D], DT, tag="w")
        nc.sync.dma_start(wo_sb, wo_t)

        # ---- projections ----
        with tc.tile_pool(name="xp", bufs=3) as xpool, \
             tc.tile_pool(name="pj", bufs=3, space="PSUM") as pj_ps:
            for phase, x_in, w_sb, b_sb in (
                ("q", xq_t, w_tiles["wq"], bq_sb),
                ("k", xk_t, w_tiles["wk"], bk_sb),
                ("v", xv_t, w_tiles["wv"], bv_sb),
            ):
                dst = qT if phase == "q" else kT
                for n in range(N_QB):
                    xblk = xpool.tile([P, E_CH, QB], DT, tag="x")
                    for e in range(E_CH):
                        nc.sync.dma_start(xblk[:, e], x_in[n, :, e])
                    if phase in ("q", "k"):
                        for m in range(M_CH):
                            ps = pj_ps.tile([P, QB], F32, tag="pj")
                            for e in range(E_CH):
                                nc.tensor.matmul(
                                    ps,
                                    lhsT=w_sb[:, e, m * P:(m + 1) * P],
                                    rhs=xblk[:, e, :],
                                    start=(e == 0), stop=(e == E_CH - 1),
                                )
                            nc.vector.tensor_scalar_add(
                                dst[:, m, n * QB:(n + 1) * QB], ps,
                                b_sb[:, m:m + 1])
                    else:
                        for sc in range(QB // P):
                            ps = pj_ps.tile([P, DH], F32, tag="pj")
                            for e in range(E_CH):
                                nc.tensor.matmul(
                                    ps,
                                    lhsT=xblk[:, e, sc * P:(sc + 1) * P],
                                    rhs=w_sb[:, e, :],
                                    start=(e == 0), stop=(e == E_CH - 1),
                                )
                            kc = n * (QB // P) + sc
                            nc.vector.tensor_tensor(
                                v_aug[:, kc, :, 0:64],
                                ps.rearrange("p (h e) -> p h e", h=NH_G),
                                bv_sb.rearrange("p (h e) -> p h e", h=NH_G),
                                ALU.add,
                            )

        if debug_dumps:
            nc.sync.dma_start(dbg_qT, qT)
            nc.sync.dma_start(dbg_kT, kT)
            nc.sync.dma_start(dbg_vaug, v_aug)

        # ---- attention + output projection ----
        with tc.tile_pool(name="sc_ps", bufs=4, space="PSUM") as sc_ps, \
             tc.tile_pool(name="pv_ps", bufs=2, space="PSUM") as pv_pool, \
             tc.tile_pool(name="rb_ps", bufs=1, space="PSUM") as rb_pool, \
             tc.tile_pool(name="op_ps", bufs=1, space="PSUM") as op_ps, \
             tc.tile_pool(name="probs", bufs=6) as probs_pool, \
             tc.tile_pool(name="attn", bufs=M_CH + 1) as attn_pool, \
             tc.tile_pool(name="misc", bufs=3) as misc, \
             tc.tile_pool(name="aupool", bufs=2 * M_CH + 1) as aupool, \
             tc.tile_pool(name="outst", bufs=3) as outst:
            for qb in range(N_QB):
                attn_tiles = []
                mq_work = []
                sums_g = misc.tile([2 * M_CH, QB], F32, tag="sums_g")
                recips_g = misc.tile([2 * M_CH, QB], F32R, tag="recips_g")
                for m in range(M_CH):
                    n_kc = (qb + 1) * (QB // KC) if causal else N_KC
                    pv_t = [pv_pool.tile([65, QB], F32, tag="pv", name=f"pv{hl}")
                             for hl in (0, 1)]
                    for kc in range(n_kc):
                        r = kc - (n_kc - R)
                        is_diag = causal and r >= 0
                        ps_h = [sc_ps.tile([P, QB], F32, tag="sc",
                                          name=f"sc{hl}")
                                for hl in (0, 1)]
                        for hl in (0, 1):
                            rows = slice(64 * hl, 64 * hl + 64)
                            nc.tensor.matmul(
                                ps_h[hl],
                                lhsT=kT[rows, m, kc * KC:(kc + 1) * KC],
                                rhs=qT[rows, m, qb * QB:(qb + 1) * QB],
                                start=True, stop=True,
                            )
                        for hl in (0, 1):
                            pt = probs_pool.tile([P, QB], DT, tag="pt")
                            c0 = KC * r if is_diag and r > 0 else 0
                            if c0:
                                nc.gpsimd.memset(pt[:, 0:c0], 0.0)
                            nc.scalar.activation(pt[:, c0:], ps_h[hl][:, c0:],
                                                 AF.Exp, scale=float(SCALE))
                            if is_diag:
                                nc.vector.tensor_tensor(
                                    pt[:, c0:], pt[:, c0:],
                                    dmask[:, r, c0:], ALU.mult)
                            if (debug_dumps and qb == 0 and m == 0
                                    and kc == 0 and hl == 0):
                                nc.sync.dma_start(dbg_probs, pt)
                            nc.tensor.matmul(
                                pv_t[hl],
                                lhsT=v_aug[:, kc, 2 * m + hl, :],
                                rhs=pt,
                                start=(kc == 0), stop=(kc == n_kc - 1),
                            )
                    # Drain PSUM fast: copy attn + sums out so PV tiles
                    # recycle; stash sums rows for the per-qb batched recip.
                    attn_us = []
                    for hl in (0, 1):
                        attn_u = aupool.tile([64, QB], DT, tag="attn_u",
                                             name=f"attn_u{hl}")
                        nc.any.tensor_copy(attn_u, pv_t[hl][0:64, :])
                        attn_us.append(attn_u)
                        sums_sb = misc.tile([65, QB], F32, tag="sums_sb",
                                            name=f"sums{hl}")
                        nc.any.tensor_copy(sums_sb[64:65, :],
                                           pv_t[hl][64:65, :])
                        nc.sync.dma_start(sums_g[2 * m + hl: 2 * m + hl + 1],
                                          sums_sb[64:65, :])
                    mq_work.append((m, attn_us))
                for m, attn_us in mq_work:
                    if m == 0:
                        # one 8-lane reciprocal for all (m, hl) of this qb
                        with nc.allow_low_precision(
                                reason="softmax denom recip, f32r rounding"):
                            nc.vector.reciprocal(recips_g, sums_g)
                    attn_m = attn_pool.tile([P, QB], DT, tag="attn")
                    for hl in (0, 1):
                        recip65 = misc.tile([65, QB], F32R, tag="recip")
                        nc.sync.dma_start(recip65[64:65, :],
                                          recips_g[2 * m + hl: 2 * m + hl + 1])
                        rb = rb_pool.tile([64, QB], F32, tag="rb")
                        nc.tensor.matmul(rb, lhsT=ones65[64:65, :],
                                         rhs=recip65[64:65, :],
                                         start=True, stop=True)
                        nc.vector.tensor_tensor(
                            attn_m[64 * hl:64 * hl + 64, :], attn_us[hl], rb,
                            ALU.mult)
                    if debug_dumps and qb == 0:
                        nc.sync.dma_start(dbg_attn[m], attn_m)
                    attn_tiles.append(attn_m)
                for ssub in range(QB // P):
                    for nout in range(O_N):
                        pso = op_ps.tile([P, 512], F32, tag="op")
                        for m in range(M_CH):
                            nc.tensor.matmul(
                                pso,
                                lhsT=attn_tiles[m][:, ssub * P:(ssub + 1) * P],
                                rhs=wo_sb[:, m, nout * 512:(nout + 1) * 512],
                                start=(m == 0), stop=(m == M_CH - 1),
                            )
                        st = outst.tile([P, 512], F32, tag="st")
                        nc.any.tensor_copy(st, pso)
                        nc.gpsimd.dma_start(
                            out_p[qb * QB + ssub * P: qb * QB + (ssub + 1) * P,
                                  nout * 512:(nout + 1) * 512],
                            st)
    nc.compile()
    return nc


def make_consts(S, use_bf16):
    """Host-built 0/1 causal masks for the R diagonal key-chunks."""
    d = _dims(S)
    QB, R = d["QB"], d["R"]
    npdt = _np_dt(use_bf16)
    i = np.arange(P)[:, None]
    j = np.arange(QB)[None, :]
    dmask = np.stack([(i <= j - KC * r) for r in range(R)], 1)
    return dmask.astype(npdt)


def core_inputs(Q, K, V, W_q, b_q, W_k, b_k, W_v, b_v, W_o, b, hg, S, use_bf16):
    """Build the per-core input map (host-side slicing/transposition/casts)."""
    npdt = _np_dt(use_bf16)
    d = _dims(S)
    M_CH = d["M_CH"]
    rows = slice(hg * DH, (hg + 1) * DH)

    QB = d["QB"]
    E_CH = D // P

    def xt(x):
        # [S, D] -> [N_QB, P, E_CH, QB]: per-core x, transposed and tiled so
        # each device block load is a contiguous DMA.
        a = np.asarray(x, np.float32).T.astype(npdt)      # [D, S]
        a = a.reshape(E_CH, P, S // QB, QB).transpose(2, 1, 0, 3)
        return np.ascontiguousarray(a)

    def wt(w):
        # [DH, D] slice -> W^T tiled [P, E_CH, DH]
        a = np.asarray(w, np.float32).T.astype(npdt)      # [D, DH]
        return np.ascontiguousarray(
            a.reshape(E_CH, P, DH).transpose(1, 0, 2))

    def wot(w):
        # W_o[:, rows] -> [P, M_CH, D]
        a = np.asarray(w, np.float32).T.astype(npdt)      # [DH, D]... wait
        return a

    a_wo = np.asarray(W_o[:, rows], np.float32).T.astype(npdt)  # [DH, D]
    wo_prep = np.ascontiguousarray(
        a_wo.reshape(M_CH, P, D).transpose(1, 0, 2))

    dmask = make_consts(S, use_bf16)
    return {
        "xq_t": xt(Q[b]), "xk_t": xt(K[b]), "xv_t": xt(V[b]),
        "wq_t": wt(W_q[rows]), "wk_t": wt(W_k[rows]), "wv_t": wt(W_v[rows]),
        "wo_t": wo_prep,
        "bq_p": np.ascontiguousarray(
            np.asarray(b_q[rows], np.float32).reshape(M_CH, P).T),
        "bk_p": np.ascontiguousarray(
            np.asarray(b_k[rows], np.float32).reshape(M_CH, P).T),
        "bv_r": np.broadcast_to(
            np.asarray(b_v[rows], np.float32), (P, DH)).copy(),
        "dmask": dmask,
        "ones_c": np.ones((65, 64), np.float32),
        "ones_v": np.ones((P, d["N_KC"], NH_G, 1), npdt),
    }


def _np_reference(Q, K, V, mask, W_q, b_q, W_k, b_k, W_v, b_v, W_o, b_o):
    """Exact numpy fallback for arbitrary masks."""
    q = (Q @ W_q.T + b_q).reshape(B, S_FULL, H, DK).transpose(0, 2, 1, 3)
    k = (K @ W_k.T + b_k).reshape(B, S_FULL, H, DK).transpose(0, 2, 1, 3)
    v = (V @ W_v.T + b_v).reshape(B, S_FULL, H, DK).transpose(0, 2, 1, 3)
    scores = np.einsum("bhqd,bhkd->bhqk", q, k) / np.sqrt(np.float32(DK))
    scores = np.where(mask == 0, np.finfo(np.float32).min, scores)
    scores -= scores.max(-1, keepdims=True)
    probs = np.exp(scores)
    probs /= probs.sum(-1, keepdims=True)
    out = np.einsum("bhqk,bhkd->bhqd", probs, v)
    out = out.transpose(0, 2, 1, 3).reshape(B, S_FULL, D)
    return (out @ W_o.T + b_o).astype(np.float32)


def kernel(Q, K, V, mask, W_q, b_q, W_k, b_k, W_v, b_v, W_o, b_o):
    Q = np.asarray(Q, np.float32)
    K = np.asarray(K, np.float32)
    V = np.asarray(V, np.float32)
    mask = np.asarray(mask)

    m2 = mask.reshape(mask.shape[-2], mask.shape[-1])
    if np.array_equal(m2 != 0, np.tril(np.ones(m2.shape, bool))):
        causal = True
    elif (m2 != 0).all():
        causal = False
    else:
        return _np_reference(Q, K, V, mask, W_q, b_q, W_k, b_k, W_v, b_v,
                             W_o, b_o)

    use_bf16 = os.environ.get("MHA_KERNEL_DTYPE", "f32r") == "bf16"
    import concourse.bass_utils as _bu
    from concourse.bass_utils import run_bass_kernel_spmd
    if (os.environ.get("MHA_LDW_OPT", "0") == "1"
            and not getattr(_bu, "_mha_ldw_patched", False)):
        _orig_rc = _bu.run_command

        def _rc(argv, **kw):
            argv = ["--enable-ldw-opt=true" if a == "--enable-ldw-opt=false"
                    else a for a in argv]
            return _orig_rc(argv, **kw)

        _bu.run_command = _rc
        _bu._mha_ldw_patched = True

    key = (causal, S_FULL, use_bf16)
    if key not in _PROG_CACHE:
        _PROG_CACHE[key] = build_program(causal, S_FULL, use_bf16)
    nc = _PROG_CACHE[key]

    in_maps = []
    for c in range(8):
        b, hg = divmod(c, 2)
        in_maps.append(core_inputs(Q, K, V, W_q, b_q, W_k, b_k, W_v, b_v,
                                   W_o, b, hg, S_FULL, use_bf16))

    trace = os.environ.get("MHA_KERNEL_TRACE", "0") == "1"
    kw = {}
    if trace:
        kw = {"trace": True,
              "trace_cores": [int(x) for x in os.environ.get(
                  "MHA_TRACE_CORES", "0").split(",")]}
    n_cores = int(os.environ.get("MHA_CORES", "8"))
    res = run_bass_kernel_spmd(nc, in_maps[:n_cores],
                               core_ids=list(range(n_cores)), **kw)
    kernel.last_results = res

    b_o32 = np.asarray(b_o, np.float32)
    out = np.zeros((B, S_FULL, D), np.float32)
    for b in range(B):
        if 2 * b + 1 < n_cores:
            out[b] = (res.results[2 * b]["out_p"]
                      + res.results[2 * b + 1]["out_p"] + b_o32[None, :])
    return out


kernel.last_results = None
